# revision 43
# baseline (speedup 1.0000x reference)
"""Trainium2 Bass kernel for nn_Attention_54159537603130.

Dense GQA attention block (QKV proj + RoPE + causal attention + out proj),
sharded over 8 NeuronCores as (batch=2) x (kv-head groups=4).  Each core
computes a [S, DIM] partial of the output projection (wo is row-sharded);
the host sums the 4 group partials per batch.

All on-chip matmul operands live in "transposed" feature-on-partition
layouts so no large on-chip transposes are needed:
  Q^T/K^T [d, t]  -> scores^T tiles [t, s] directly
  V token-major [t, d] -> out^T = V^T @ P^T via PE accumulation
  out^T [d, s] is exactly the lhsT of the wo matmul.

Softmax runs without max-subtraction (logits are O(10) here).  The softmax
denominator is accumulated on the Pool engine (sum of exp tiles into an
f32 SBUF tile) so the PE only spends one ones-matmul per (head, query
block) on it; 1/r = exp(-ln r) on ScalarE and the normalization multiply
reads r^-1 through a partition-broadcast (stride-0) AP on the DVE.

The attention phase (ACT-bound: one big Exp per score tile) and the output
projection (pure PE) are software-pipelined: wo-projection matmuls for
query block j-1 are interleaved into the attention instruction stream of
block j, so the tensor engine stays busy while ScalarE works through the
exps.  Causality is exploited at 128-column granularity on the diagonal
tiles (shortened QK/exp/PV + one [128,128] triangle mask).
"""

import os
import sys

sys.path.insert(0, "/opt/trn_rl_repo")

import numpy as np
import ml_dtypes

import concourse.bass as bass
import concourse.tile as tile
from concourse import mybir

BF16 = mybir.dt.bfloat16
F32 = mybir.dt.float32
NPBF16 = ml_dtypes.bfloat16

DIM, NH, NKV, HD = 4096, 32, 8, 128
B, S = 2, 2048
NCORES = 8
GQ = 8  # q heads per core
GKV = 2  # kv heads per core
MQ = GQ * HD  # 1024 q-proj cols per core
MKV = GKV * HD  # 256 kv-proj cols per core
SC = 1.0 / np.sqrt(HD)
NEG_INF = -1e9

LAST_EXEC_TIME_NS = None
LAST_RESULTS = None


def _install_ntff_hook():
    """antenv.axon_hooks is absent in this image; reconstruct the NTFF
    profiling hook via ctypes against libaxon_pjrt.so (only used when
    BASS_TRACE is set)."""
    import types
    import contextlib
    import ctypes

    if "antenv.axon_hooks" in sys.modules:
        return
    try:
        lib = ctypes.CDLL("/opt/axon/libaxon_pjrt.so")
        have = hasattr(lib, "axon_start_nrt_profile")
    except OSError:
        have = False

    if have:
        lib.axon_start_nrt_profile.argtypes = [
            ctypes.POINTER(ctypes.c_int64),
            ctypes.c_size_t,
        ]
        lib.axon_start_nrt_profile.restype = ctypes.c_int64
        lib.axon_stop_nrt_profile.argtypes = [ctypes.c_char_p]
        lib.axon_stop_nrt_profile.restype = ctypes.c_int64

        @contextlib.contextmanager
        def _hook(output_dir, device_ids):
            import jax

            jax.devices()
            if device_ids:
                ids = (ctypes.c_int64 * len(device_ids))(*device_ids)
                rc = lib.axon_start_nrt_profile(ids, len(device_ids))
            else:
                rc = lib.axon_start_nrt_profile(None, 0)
            if rc != 0:
                raise RuntimeError(f"axon_start_nrt_profile rc={rc}")
            try:
                yield
            finally:
                n = lib.axon_stop_nrt_profile(str(output_dir).encode())
                print(f"profile: {n} file(s) written to {output_dir}")

        hook = _hook
    else:
        hook = None

    mod = types.ModuleType("antenv.axon_hooks")
    mod.get_axon_ntff_profile_hook = lambda: hook
    mod.set_axon_ntff_profile_hook = lambda h: None
    sys.modules["antenv.axon_hooks"] = mod


def split_excess_waits(nc, max_waits=1):
    """walrus codegen supports very few sync waits per instruction while
    Tile's tail/release drains can carry several; hoist excess onto NOPs."""
    for fn in nc.m.functions:
        for blk in fn.blocks:
            insts = blk.instructions
            changed = False
            i = 0
            while i < len(insts):
                inst = insts[i]
                si = inst.sync_info
                if (
                    si is not None
                    and si.on_wait is not None
                    and len(si.on_wait) > max_waits
                ):
                    w = si.on_wait
                    k = 0
                    while len(w) > max_waits:
                        nop = mybir.InstNoOp(
                            name=f"{inst.name}_wsplit{k}",
                            engine=inst.engine,
                            ins=[],
                            outs=[],
                        )
                        nop.sync_info = mybir.SyncInfo(
                            on_wait=w[:max_waits], on_update=[]
                        )
                        insts.insert(i, nop)
                        i += 1
                        w = w[max_waits:]
                        k += 1
                    inst.sync_info = mybir.SyncInfo(on_wait=w, on_update=si.on_update)
                    changed = True
                i += 1
            if changed:
                blk.instructions = insts


def _pump(gen, k):
    """Advance an (optional) interleaved-emission generator k steps."""
    if gen is None:
        return
    for _ in range(k):
        try:
            next(gen)
        except StopIteration:
            return


def _build_causal():
    nc = bass.Bass("TRN2", target_bir_lowering=False, debug=False)
    Exp = mybir.ActivationFunctionType.Exp
    Ln = mybir.ActivationFunctionType.Ln

    # DRAM I/O — all inputs pre-tiled on the host into SBUF-friendly
    # [partition, ...] layouts with large contiguous per-partition runs.
    xq_d = nc.dram_tensor("xq", [4, 128, 32, 512], BF16, kind="ExternalInput").ap()
    wq_d = nc.dram_tensor("wq", [8, 128, 32, 128], BF16, kind="ExternalInput").ap()
    wk_d = nc.dram_tensor("wk", [2, 128, 32, 128], BF16, kind="ExternalInput").ap()
    wv_d = nc.dram_tensor("wv", [128, 32, 256], BF16, kind="ExternalInput").ap()
    wo_d = nc.dram_tensor("wo", [8, 128, 4096], BF16, kind="ExternalInput").ap()
    cek_d = nc.dram_tensor("cek", [128, 2048], BF16, kind="ExternalInput").ap()
    s2k_d = nc.dram_tensor("s2k", [128, 2048], BF16, kind="ExternalInput").ap()
    psw_d = nc.dram_tensor("pswap", [128, 128], BF16, kind="ExternalInput").ap()
    tri_d = nc.dram_tensor("tri", [128, 128], F32, kind="ExternalInput").ap()
    out_d = nc.dram_tensor("out", [128, 16, 4096], F32, kind="ExternalOutput").ap()

    with tile.TileContext(nc) as tc:
        # ---- pools ---------------------------------------------------
        # consts/pa/aps are released after the projection phase; persist,
        # cb, outp, pb, bps live through attention; pwo/cps/pc are
        # allocated after the release so SBUF/PSUM budgets stay legal.
        # (pools release in LIFO order: the A-phase pools are allocated
        # last so they can be popped first.)
        cb = tc.alloc_tile_pool(name="cb", bufs=1)
        persist = tc.alloc_tile_pool(name="persist", bufs=1)
        outp = tc.alloc_tile_pool(name="outp", bufs=1)
        pb = tc.alloc_tile_pool(name="pb", bufs=1)
        bps = tc.alloc_tile_pool(name="bps", bufs=1, space="PSUM")
        consts = tc.alloc_tile_pool(name="consts", bufs=1)
        pa = tc.alloc_tile_pool(name="pa", bufs=1)
        aps = tc.alloc_tile_pool(name="aps", bufs=1, space="PSUM")

        # ---- startup DMAs, ordered by first use ----------------------
        # Two independent trigger queues: the sync engine carries the
        # critical path (first x sub-tile, first weights, Q-rope consts,
        # then the per-m weight stream emitted inside the loops), while
        # gpsimd (idle during the projection phase) prefetches the bulk
        # loads so they never sit in front of an urgent transfer.
        def load_xh(q, eng, gate=False):
            parts = []
            for p in range(4):
                t = pa.tile(
                    [128, 8, 512], BF16, tag=f"xh{p}", bufs=2, name=f"xh_{q}_{p}"
                )
                if gate:  # q>=2 is WAR-gated naturally by slot reuse
                    nc.vector.tensor_copy(out=t[0:1, 0, 0:2], in_=gate_src)
                eng.dma_start(out=t, in_=xq_d[q, :, 8 * p : 8 * p + 8])
                parts.append(t)
            return parts

        # The DMA hardware services ALL queued descriptors roughly fairly
        # (even within one queue), so anything queued early delays the
        # critical first loads.  Prefetch in gated tiers: a tiny copy
        # into each destination tile (depending on the previous tier)
        # forces the DMA — a second writer of the tile — to wait.  A bare
        # ordering hint wouldn't survive the tile scheduler's reordering.
        def gated_dma(eng, out_ap, gate_slice, in_ap, gate_from):
            if gate_from is not None:
                # DVE: a tiny gpsimd copy is a ~2us CAST custom op
                nc.vector.tensor_copy(out=gate_slice, in_=gate_from)
            eng.dma_start(out=out_ap, in_=in_ap)

        # tier 1: first x sub-tile + first weight tile, nothing else
        xh0 = []
        for p in range(4):
            t = pa.tile([128, 8, 512], BF16, tag=f"xh{p}", bufs=2, name=f"xh_0_{p}")
            xh0.append(t)
        nc.sync.dma_start(out=xh0[0], in_=xq_d[0, :, 0:8])
        wqc0 = pa.tile([128, 32, 128], BF16, tag="wc", bufs=3)
        nc.sync.dma_start(out=wqc0, in_=wq_d[0])
        t1_src = wqc0[0:1, 0, 0:2]
        # warm the PE p-state during the initial DMA wait: ~60 junk
        # matmuls on the memset ones tile (into the psr bank, which has
        # no real user until B0) so the first projection chains run at
        # full clock instead of ramping through them
        # tier 2: remaining x sub-tiles, gated on the first weights
        for p in (1, 2, 3):
            gated_dma(
                nc.gpsimd, xh0[p], xh0[p][0:1, 0, 0:2], xq_d[0, :, 8 * p : 8 * p + 8],
                t1_src,
            )
        # tier 4: rope consts (first needed ~15us in), gated on tier 3
        # (wq is pre-scaled by 1/sqrt(HD) on the host, so Q and K share
        # one set of rope planes)
        t4_src = xh0[2][0:1, 0, 0:2]
        psw_t = consts.tile([128, 128], BF16)
        gated_dma(nc.gpsimd, psw_t, psw_t[0:1, 0:2], psw_d, t4_src)
        cek_t = consts.tile([128, 2048], BF16)
        gated_dma(nc.gpsimd, cek_t, cek_t[0:1, 0:2], cek_d, t4_src)
        s2k_t = consts.tile([128, 2048], BF16)
        gated_dma(nc.gpsimd, s2k_t, s2k_t[0:1, 0:2], s2k_d, t4_src)
        # tier 5 (gated on the last x sub-tile): wv, mask
        gate_src = xh0[3][0:1, 0, 0:2]
        wv_t = pa.tile([128, 32, 256], BF16, tag="wv", bufs=1)
        gated_dma(nc.gpsimd, wv_t, wv_t[0:1, 0, 0:2], wv_d, gate_src)
        tri_t = cb.tile([128, 128], F32)
        gated_dma(nc.gpsimd, tri_t, tri_t[0:1, 0:2], tri_d, gate_src)
        # all-ones [128,128]: the rowsum matmul then yields the softmax
        # denominator already replicated across all output partitions
        ones_mat = cb.tile([128, 128], BF16)
        nc.vector.memset(ones_mat, 1.0)
        # warm the PE p-state during the initial DMA wait: ~60 junk
        # matmuls on the memset ones tile (into the psr bank, which has
        # no real user until B0) so the first projection chains run at
        # full clock instead of ramping through them
        warm_ps = bps.tile([128, 512], F32, tag="psr", bufs=1, name="warm")
        for w_ in range(60):
            nc.tensor.matmul(
                out=warm_ps[:, 0:128], lhsT=ones_mat, rhs=ones_mat,
                start=True, stop=True,
            )

        qrot = persist.tile([128, GQ, 2048], BF16)
        krot = persist.tile([128, GKV, 2048], BF16)
        vtok = persist.tile([128, 16, MKV], BF16)

        # ---------------- attention row emitter -----------------------
        def b_row(j, h, outT_j, cgen, cper, pending):
            """Emit one head-row of attention for query block j.

            Tile order: diagonal t0 first (full width, carries the PSUM
            accumulation start), then the off-diagonal tiles, then the
            shortened diagonal tiles.  QK is emitted one tile ahead of
            PV, and cgen (the wo-projection of block j-1) is pumped
            between tiles to keep the PE busy while ScalarE exps.  The
            exp-sum (softmax denominator) is accumulated in f32 off the
            PE: tiles alternate between the DVE and Pool engines so
            neither serial chain falls behind the row.
            """
            kv = h // 4
            s0 = 512 * j
            # (key tile index, first valid column, is_diag)
            tiles = [(4 * j, 0, True)]
            tiles += [(i, 0, False) for i in range(4 * j)]
            tiles += [(4 * j + t, 128 * t, True) for t in range(1, 4)]
            n = len(tiles)
            pso = bps.tile([128, 512], F32, tag="pso", bufs=2)
            # bf16 accumulation on the DVE (2x rate): costs ~0.4% on the
            # softmax denominator, well inside the error budget, and keeps
            # the per-row reduction chain far off the critical path
            es16 = pb.tile([128, 512], BF16, tag="es16", bufs=2)

            def emit_qk(idx):
                i, c0, diag = tiles[idx]
                pss = bps.tile([128, 512], F32, tag="pss", bufs=2)
                nc.tensor.matmul(
                    out=pss[:, c0:512],
                    lhsT=krot[:, kv, 128 * i : 128 * i + 128],
                    rhs=qrot[:, h, s0 + c0 : s0 + 512],
                    start=True,
                    stop=True,
                )
                return pss

            def emit_pv(idx, e):
                i, c0, diag = tiles[idx]
                nc.tensor.matmul(
                    out=pso[:, c0:512],
                    lhsT=vtok[:, i, 128 * kv : 128 * kv + 128],
                    rhs=e[:, c0:512],
                    start=(idx == 0),
                    stop=(idx == n - 1),
                )

            pss_cur = emit_qk(0)
            pv_lag = None  # (idx, e) of the tile whose PV is not yet emitted
            for idx in range(n):
                pss_next = emit_qk(idx + 1) if idx + 1 < n else None
                if idx == 1 and pending is not None:
                    # previous row's finalize, deferred so its blocksum
                    # matmul never makes the PE wait on the esum chains
                    pending()
                    pending = None
                i, c0, diag = tiles[idx]
                if diag:  # triangle mask on the 128 partially-valid cols
                    nc.vector.tensor_add(
                        pss_cur[:, c0 : c0 + 128], pss_cur[:, c0 : c0 + 128], tri_t
                    )
                e = pb.tile([128, 512], BF16, tag="e", bufs=6)
                nc.scalar.activation(out=e[:, c0:512], in_=pss_cur[:, c0:512], func=Exp)
                if idx == 0:
                    nc.vector.tensor_copy(out=es16, in_=e)
                else:
                    nc.vector.tensor_add(
                        es16[:, c0:512], es16[:, c0:512], e[:, c0:512]
                    )
                # PV runs one tile behind its exp for extra handoff slack
                if pv_lag is not None:
                    emit_pv(*pv_lag)
                pv_lag = (idx, e)
                pss_cur = pss_next
                _pump(cgen, cper)
            emit_pv(*pv_lag)

            if pending is not None:
                pending()

            # softmax denominator + normalization, deferred into the next
            # row's instruction stream
            def finalize():
                psr = bps.tile([128, 512], F32, tag="psr", bufs=1)
                nc.tensor.matmul(
                    out=psr, lhsT=ones_mat, rhs=es16, start=True, stop=True
                )
                lnr = pb.tile([128, 512], F32, tag="lnr", bufs=2)
                nc.scalar.activation(out=lnr, in_=psr, func=Ln)
                rp = pb.tile([128, 512], F32, tag="rp", bufs=2)
                nc.scalar.activation(out=rp, in_=lnr, func=Exp, scale=-1.0)
                nc.vector.tensor_mul(outT_j[:, h, :], pso, rp)

            return finalize

        def b_block_gen(j, outT_j):
            """Generator form of one attention block (yields per row)."""
            pending = None
            for h in range(GQ):
                pending = b_row(j, h, outT_j, None, 0, pending)
                yield
            if pending is not None:
                pending()

        # ---------------- projection phase (A) ------------------------
        # rope is split: the PSUM->SBUF copy is emitted right after the
        # projection chain, but the pair-swap matmul (which must wait for
        # that ACT copy) is deferred into the NEXT chain's matmul stream
        # so it never blocks the PE queue head.
        def rope(ps, ce, s2, dst, toff):
            qb = pa.tile([128, 512], BF16, tag="ropeb", bufs=2)
            nc.scalar.copy(out=qb, in_=ps)

            def fin():
                sw = aps.tile([128, 512], F32, tag="swvp", bufs=1)
                nc.tensor.matmul(out=sw, lhsT=psw_t, rhs=qb, start=True, stop=True)
                a = pa.tile([128, 512], BF16, tag="ropea", bufs=2)
                nc.vector.tensor_mul(a, qb, ce[:, toff : toff + 512])
                bt = pa.tile([128, 512], BF16, tag="ropec", bufs=2)
                nc.vector.tensor_mul(bt, sw, s2[:, toff : toff + 512])
                nc.vector.tensor_add(dst, a, bt)

            return fin

        outT0 = outp.tile([128, GQ, 512], BF16, tag="outT", bufs=2)
        b0_gen = None
        rope_fin = None

        def fire_rope():
            nonlocal rope_fin
            if rope_fin is not None:
                rope_fin()
                rope_fin = None

        for q in range(4):
            t0 = 512 * q
            xh = xh0 if q == 0 else load_xh(q, nc.gpsimd, gate=(q == 1))
            if q == 3:
                b0_gen = b_block_gen(0, outT0)
            for m in range(GQ):
                if q == 0 and m == 0:
                    wqc = wqc0
                else:
                    wqc = pa.tile([128, 32, 128], BF16, tag="wc", bufs=3)
                    if q == 0 and m == 1:  # later ones WAR-gate on slot reuse
                        nc.vector.tensor_copy(
                            out=wqc[0:1, 0, 0:2], in_=xh0[1][0:1, 0, 0:2]
                        )
                    nc.sync.dma_start(out=wqc, in_=wq_d[m])
                ps = aps.tile([128, 512], F32, tag="proj", bufs=2)
                for d in range(32):
                    nc.tensor.matmul(
                        out=ps,
                        lhsT=wqc[:, d],
                        rhs=xh[d // 8][:, d % 8],
                        start=(d == 0),
                        stop=(d == 31),
                    )
                    if d == 5:
                        fire_rope()
                rope_fin = rope(ps, cek_t, s2k_t, qrot[:, m, t0 : t0 + 512], t0)
                _pump(b0_gen, 1)
            for m in range(GKV):
                wkc = pa.tile([128, 32, 128], BF16, tag="wc", bufs=3)
                nc.sync.dma_start(out=wkc, in_=wk_d[m])
                ps = aps.tile([128, 512], F32, tag="proj", bufs=2)
                for d in range(32):
                    nc.tensor.matmul(
                        out=ps,
                        lhsT=wkc[:, d],
                        rhs=xh[d // 8][:, d % 8],
                        start=(d == 0),
                        stop=(d == 31),
                    )
                    if d == 5:
                        fire_rope()
                rope_fin = rope(ps, cek_t, s2k_t, krot[:, m, t0 : t0 + 512], t0)
            for tv in range(4):
                # proj tag (bufs=2): V chain tv+1 overlaps the vtok copy of
                # chain tv; the single swvp buffer would serialize them
                psv = aps.tile([128, 512], F32, tag="proj", bufs=2)
                for d in range(32):
                    nc.tensor.matmul(
                        out=psv[:, 0:256],
                        lhsT=xh[d // 8][:, d % 8, 128 * tv : 128 * tv + 128],
                        rhs=wv_t[:, d],
                        start=(d == 0),
                        stop=(d == 31),
                    )
                    if d == 5:
                        fire_rope()
                nc.scalar.copy(out=vtok[:, 4 * q + tv, :], in_=psv[:, 0:256])
        fire_rope()
        _pump(b0_gen, GQ)  # drain any B0 rows not covered by the A3 loop

        # projection-phase pools are done: release them, then prefetch wo
        # (split per m so block-0 wo matmuls don't wait on the full 8MB).
        aps.release()
        pa.release()
        consts.release()

        pwo = tc.alloc_tile_pool(name="pwo", bufs=1)
        wo_m = []
        for m in range(8):
            w = pwo.tile([128, 4096], BF16, tag=f"wo{m}", bufs=1)
            # chain-gated loads (m on m-1, m=0 on B0's first output): each
            # 1MB tile lands alone in ~3.5us, in consumption order, rather
            # than the whole 8MB fair-sharing the DMA hardware and landing
            # together after C0 already needs wo_m[0]
            gsrc = outT0[0:1, 0, 0:2] if m == 0 else wo_m[m - 1][0:1, 0:2]
            nc.vector.tensor_copy(out=w[0:1, 0:2], in_=gsrc)
            nc.gpsimd.dma_start(out=w, in_=wo_d[m])
            wo_m.append(w)
        cps = tc.alloc_tile_pool(name="cps", bufs=1, space="PSUM")
        pc = tc.alloc_tile_pool(name="pc", bufs=1)

        # ---------------- wo projection emitter (C) -------------------
        def c_gen(j, outT_j):
            """Generator: output projection for query block j; yields in
            ~3-matmul steps so it can interleave into block j+1's
            attention stream."""
            for s4 in range(4):
                s = 4 * j + s4
                # chunks of 2 against a 3-slot psf pool: chunk n+1's
                # matmuls overlap chunk n's PSUM->SBUF copies; the very
                # last tile uses single-dc chunks so the final copy+DMA
                # tail is short
                last = j == 3 and s4 == 3
                chunks = (
                    tuple((dc,) for dc in range(8))
                    if last
                    else ((0, 1), (2, 3), (4, 5), (6, 7))
                )
                for chunk in chunks:
                    psfs = [
                        cps.tile(
                            [128, 512], F32, tag="psf", bufs=3,
                            name=f"psf_{j}_{s4}_{c0_}_{k_}",
                        )
                        for k_, c0_ in enumerate(chunk)
                    ]
                    for m in range(8):
                        for k in range(len(chunk)):
                            nc.tensor.matmul(
                                out=psfs[k],
                                lhsT=outT_j[:, m, 128 * s4 : 128 * s4 + 128],
                                rhs=wo_m[m][:, 512 * chunk[k] : 512 * chunk[k] + 512],
                                start=(m == 0),
                                stop=(m == 7),
                            )
                        yield
                    ot = pc.tile([128, 512 * len(chunk)], F32, tag="ot", bufs=2)
                    for k in range(len(chunk)):
                        nc.scalar.copy(
                            out=ot[:, 512 * k : 512 * (k + 1)], in_=psfs[k]
                        )
                    nc.sync.dma_start(
                        out=out_d[
                            :, s, 512 * chunk[0] : 512 * chunk[0] + 512 * len(chunk)
                        ],
                        in_=ot,
                    )
                    yield

        # ---------------- attention blocks 1..3 + interleaved wo ------
        outT_prev, cgen, pending = outT0, None, None
        for j in range(1, 4):
            cgen = c_gen(j - 1, outT_prev)
            cper = 2 if j < 3 else 1
            outT_j = outp.tile([128, GQ, 512], BF16, tag="outT", bufs=2)
            for h in range(GQ):
                pending = b_row(j, h, outT_j, cgen, cper, pending)
            _pump(cgen, 10**6)  # drain leftovers before the next block
            outT_prev = outT_j
        if pending is not None:
            pending()
        cgen = c_gen(3, outT_prev)
        _pump(cgen, 10**6)

        pc.release()
        cps.release()
        pwo.release()
        bps.release()
        pb.release()
        outp.release()
        persist.release()
        cb.release()

    return nc


def _build_fallback(causal: bool):
    """Baseline kernel (used for the non-causal general-mask path)."""
    nc = bass.Bass("TRN2", target_bir_lowering=False, debug=False)
    Exp = mybir.ActivationFunctionType.Exp

    xq_d = nc.dram_tensor("xq", [4, 128, 32, 512], BF16, kind="ExternalInput").ap()
    wq_d = nc.dram_tensor("wq", [8, 128, 32, 128], BF16, kind="ExternalInput").ap()
    wk_d = nc.dram_tensor("wk", [2, 128, 32, 128], BF16, kind="ExternalInput").ap()
    wv_d = nc.dram_tensor("wv", [128, 32, 256], BF16, kind="ExternalInput").ap()
    wo_d = nc.dram_tensor("wo", [128, 8, 4096], BF16, kind="ExternalInput").ap()
    ceq_d = nc.dram_tensor("ceq", [128, 2048], BF16, kind="ExternalInput").ap()
    s2q_d = nc.dram_tensor("s2q", [128, 2048], BF16, kind="ExternalInput").ap()
    cek_d = nc.dram_tensor("cek", [128, 2048], BF16, kind="ExternalInput").ap()
    s2k_d = nc.dram_tensor("s2k", [128, 2048], BF16, kind="ExternalInput").ap()
    psw_d = nc.dram_tensor("pswap", [128, 128], BF16, kind="ExternalInput").ap()
    if causal:
        mask_d = nc.dram_tensor(
            "maskd", [128, 16, 512], BF16, kind="ExternalInput"
        ).ap()
    else:
        mask_d = nc.dram_tensor(
            "maskt", [128, 16, 2048], BF16, kind="ExternalInput"
        ).ap()
    out_d = nc.dram_tensor("out", [128, 16, 4096], F32, kind="ExternalOutput").ap()

    with tile.TileContext(nc) as tc:
        with (
            tc.tile_pool(name="consts", bufs=1) as consts,
            tc.tile_pool(name="persist", bufs=1) as persist,
        ):
            ceq_t = consts.tile([128, 2048], BF16)
            nc.gpsimd.dma_start(out=ceq_t, in_=ceq_d)
            s2q_t = consts.tile([128, 2048], BF16)
            nc.gpsimd.dma_start(out=s2q_t, in_=s2q_d)
            cek_t = consts.tile([128, 2048], BF16)
            nc.gpsimd.dma_start(out=cek_t, in_=cek_d)
            s2k_t = consts.tile([128, 2048], BF16)
            nc.gpsimd.dma_start(out=s2k_t, in_=s2k_d)
            psw_t = consts.tile([128, 128], BF16)
            nc.gpsimd.dma_start(out=psw_t, in_=psw_d)
            ones_col = consts.tile([128, 1], BF16)
            nc.vector.memset(ones_col, 1.0)
            ones_row = consts.tile([1, 128], F32)
            nc.vector.memset(ones_row, 1.0)

            qrot = persist.tile([128, GQ, 2048], BF16)
            krot = persist.tile([128, GKV, 2048], BF16)
            vtok = persist.tile([128, 16, MKV], BF16)
            if causal:
                mask_t = persist.tile([128, 16, 512], BF16)
                nc.gpsimd.dma_start(out=mask_t, in_=mask_d)

            with (
                tc.tile_pool(name="p1", bufs=1) as p1,
                tc.tile_pool(name="p1ps", bufs=1, space="PSUM") as pps,
            ):
                wv_t = p1.tile([128, 32, 256], BF16, tag="wv", bufs=1)
                nc.gpsimd.dma_start(out=wv_t, in_=wv_d)

                def rope(ps, ce, s2, dst, toff):
                    qb = p1.tile([128, 512], BF16, tag="ropeb", bufs=3)
                    nc.scalar.copy(out=qb, in_=ps)
                    sw = pps.tile([128, 512], F32, tag="swap", bufs=2)
                    nc.tensor.matmul(out=sw, lhsT=psw_t, rhs=qb, start=True, stop=True)
                    a = p1.tile([128, 512], BF16, tag="ropea", bufs=3)
                    nc.vector.tensor_mul(a, qb, ce[:, toff : toff + 512])
                    bt = p1.tile([128, 512], BF16, tag="ropec", bufs=3)
                    nc.vector.tensor_mul(bt, sw, s2[:, toff : toff + 512])
                    nc.vector.tensor_add(dst, a, bt)

                for q in range(4):
                    t0 = 512 * q
                    xh = p1.tile([128, 32, 512], BF16, tag="xh", bufs=2)
                    nc.gpsimd.dma_start(out=xh, in_=xq_d[q])
                    for m in range(GQ):
                        wqc = p1.tile([128, 32, 128], BF16, tag="wc", bufs=3)
                        nc.gpsimd.dma_start(out=wqc, in_=wq_d[m])
                        ps = pps.tile([128, 512], F32, tag="proj", bufs=2)
                        for d in range(32):
                            nc.tensor.matmul(
                                out=ps,
                                lhsT=wqc[:, d],
                                rhs=xh[:, d],
                                start=(d == 0),
                                stop=(d == 31),
                            )
                        rope(ps, cek_t, s2k_t, qrot[:, m, t0 : t0 + 512], t0)
                    for m in range(GKV):
                        wkc = p1.tile([128, 32, 128], BF16, tag="wc", bufs=3)
                        nc.gpsimd.dma_start(out=wkc, in_=wk_d[m])
                        ps = pps.tile([128, 512], F32, tag="proj", bufs=2)
                        for d in range(32):
                            nc.tensor.matmul(
                                out=ps,
                                lhsT=wkc[:, d],
                                rhs=xh[:, d],
                                start=(d == 0),
                                stop=(d == 31),
                            )
                        rope(ps, cek_t, s2k_t, krot[:, m, t0 : t0 + 512], t0)
                    for tv in range(4):
                        psv = pps.tile([128, 256], F32, tag="vproj", bufs=2)
                        for d in range(32):
                            nc.tensor.matmul(
                                out=psv,
                                lhsT=xh[:, d, 128 * tv : 128 * tv + 128],
                                rhs=wv_t[:, d],
                                start=(d == 0),
                                stop=(d == 31),
                            )
                        nc.scalar.copy(out=vtok[:, 4 * q + tv, :], in_=psv)

            outT_pool = tc.alloc_tile_pool(name="po", bufs=1)
            outT = outT_pool.tile([128, GQ, 2048], BF16)
            wo_pool = tc.alloc_tile_pool(name="pwo", bufs=1)
            wo_t = wo_pool.tile([128, 8, 4096], BF16)
            nc.gpsimd.dma_start(out=wo_t, in_=wo_d)

            with (
                tc.tile_pool(name="p2", bufs=1) as p2,
                tc.tile_pool(name="p2ps", bufs=1, space="PSUM") as pps2,
            ):
                if not causal:
                    mask_t = p2.tile([128, 16, 2048], BF16)
                    nc.gpsimd.dma_start(out=mask_t, in_=mask_d)

                def finalize(fin):
                    pso_, psr_, h_, s0_ = fin
                    nc.scalar.activation(
                        out=psr_,
                        in_=psr_,
                        func=mybir.ActivationFunctionType.Ln,
                    )
                    rp = p2.tile([1, 512], F32, tag="rp", bufs=2)
                    nc.scalar.activation(
                        out=rp,
                        in_=psr_,
                        func=mybir.ActivationFunctionType.Exp,
                        scale=-1.0,
                    )
                    psb = pps2.tile([128, 512], F32, tag="psb", bufs=1)
                    nc.tensor.matmul(
                        out=psb, lhsT=ones_row, rhs=rp, start=True, stop=True
                    )
                    rb = p2.tile([128, 512], F32, tag="rb", bufs=2)
                    nc.vector.tensor_copy(out=rb, in_=psb)
                    nc.vector.tensor_mul(outT[:, h_, s0_ : s0_ + 512], pso_, rb)

                pending = None
                for h in range(GQ):
                    kv = h // 4
                    for j in range(4):
                        s0 = 512 * j
                        ilist = list(range(4 * (j + 1))) if causal else list(range(16))
                        n_i = len(ilist)
                        pso = pps2.tile([128, 512], F32, tag="pso", bufs=2)
                        psr = pps2.tile([1, 512], F32, tag="psr", bufs=2)
                        for idx, i in enumerate(ilist):
                            pss = pps2.tile([128, 512], F32, tag="pss", bufs=3)
                            nc.tensor.matmul(
                                out=pss,
                                lhsT=krot[:, kv, 128 * i : 128 * i + 128],
                                rhs=qrot[:, h, s0 : s0 + 512],
                                start=True,
                                stop=True,
                            )
                            if causal:
                                if i >= 4 * j:
                                    nc.vector.tensor_add(pss, pss, mask_t[:, i, :])
                            else:
                                nc.vector.tensor_add(
                                    pss, pss, mask_t[:, i, s0 : s0 + 512]
                                )
                            e = p2.tile([128, 512], BF16, tag="exp", bufs=6)
                            nc.scalar.activation(out=e, in_=pss, func=Exp)
                            nc.tensor.matmul(
                                out=pso,
                                lhsT=vtok[:, i, 128 * kv : 128 * kv + 128],
                                rhs=e,
                                start=(idx == 0),
                                stop=(idx == n_i - 1),
                            )
                            nc.tensor.matmul(
                                out=psr[0:1, :],
                                lhsT=ones_col,
                                rhs=e,
                                start=(idx == 0),
                                stop=(idx == n_i - 1),
                            )
                            if idx == 0 and pending is not None:
                                finalize(pending)
                                pending = None
                        if pending is not None:
                            finalize(pending)
                        pending = (pso, psr, h, s0)
                finalize(pending)

            with (
                tc.tile_pool(name="p3", bufs=1) as p3,
                tc.tile_pool(name="p3ps", bufs=1, space="PSUM") as pps3,
            ):
                for s in range(16):
                    psfs = [
                        pps3.tile(
                            [128, 512], F32, tag="psf", bufs=8, name=f"psf_{s}_{dc}"
                        )
                        for dc in range(8)
                    ]
                    for m in range(8):
                        for dc in range(8):
                            nc.tensor.matmul(
                                out=psfs[dc],
                                lhsT=outT[:, m, 128 * s : 128 * s + 128],
                                rhs=wo_t[:, m, 512 * dc : 512 * dc + 512],
                                start=(m == 0),
                                stop=(m == 7),
                            )
                    for dc in range(8):
                        ot = p3.tile([128, 512], F32, tag="ot", bufs=8)
                        nc.scalar.copy(out=ot, in_=psfs[dc])
                        nc.gpsimd.dma_start(
                            out=out_d[:, s, 512 * dc : 512 * dc + 512], in_=ot
                        )

            wo_pool.release()
            outT_pool.release()

    return nc


def _prep_inputs(x, freqs_cos, freqs_sin, mask, wq, wk, wv, wo, causal):
    """Host-side shard + retile into the DMA layouts declared in _build."""
    f32 = np.float32

    # RoPE planes [128, 2048]: ce[2i,t]=ce[2i+1,t]=cos[t,i];
    # s2[2i,t]=-sin[t,i], s2[2i+1,t]=+sin[t,i]. Query planes carry 1/sqrt(HD).
    cos_t = np.asarray(freqs_cos, f32).T  # [64, 2048]
    sin_t = np.asarray(freqs_sin, f32).T
    ce = np.repeat(cos_t, 2, axis=0)  # [128, 2048]
    s2 = np.empty((HD, S), f32)
    s2[0::2] = -sin_t
    s2[1::2] = sin_t
    ceq = (ce * SC).astype(NPBF16)
    s2q = (s2 * SC).astype(NPBF16)
    cek = ce.astype(NPBF16)
    s2k = s2.astype(NPBF16)

    # pair-swap permutation
    psw = np.zeros((HD, HD), NPBF16)
    idx = np.arange(HD)
    psw[idx ^ 1, idx] = 1

    if causal:
        # triangle mask [key 128, col 128]: 0 where col >= key else -1e9
        tri = np.where(
            np.arange(128)[None, :] >= np.arange(128)[:, None], 0.0, NEG_INF
        ).astype(f32)
        mask_extra = {"tri": tri}
    else:
        maskT = np.ascontiguousarray(np.asarray(mask, f32).T)  # [t, s]
        mask_extra = {
            "maskt": np.ascontiguousarray(
                maskT.reshape(16, 128, 2048).transpose(1, 0, 2)
            ).astype(NPBF16)
        }

    # per-batch x tiles [4, 128, 32, 512]
    xq_b = []
    for b in range(B):
        xT = np.asarray(x[b], f32).astype(NPBF16).T  # [4096, 2048]
        xq_b.append(
            np.ascontiguousarray(
                xT.reshape(32, 128, 4, 512).transpose(2, 1, 0, 3)
            )
        )

    # per-group weight tiles
    wq_g, wk_g, wv_g, wo_g = [], [], [], []
    for g in range(4):
        wqs = np.asarray(wq[:, MQ * g : MQ * (g + 1)], f32)
        if causal:
            wqs = wqs * SC  # fold 1/sqrt(HD) into wq: Q and K share rope planes
        wqs = wqs.astype(NPBF16)
        wq_g.append(
            np.ascontiguousarray(wqs.reshape(32, 128, 8, 128).transpose(2, 1, 0, 3))
        )
        wks = np.asarray(wk[:, MKV * g : MKV * (g + 1)], f32).astype(NPBF16)
        wk_g.append(
            np.ascontiguousarray(wks.reshape(32, 128, 2, 128).transpose(2, 1, 0, 3))
        )
        wvs = np.asarray(wv[:, MKV * g : MKV * (g + 1)], f32).astype(NPBF16)
        wv_g.append(np.ascontiguousarray(wvs.reshape(32, 128, 256).transpose(1, 0, 2)))
        wos = np.asarray(wo[MQ * g : MQ * (g + 1), :], f32).astype(NPBF16)
        wot = np.ascontiguousarray(wos.reshape(8, 128, 4096))
        if causal:
            wo_g.append(wot)  # [m, 128, 4096]
        else:
            wo_g.append(np.ascontiguousarray(wot.transpose(1, 0, 2)))

    in_maps = []
    for c in range(NCORES):
        b, g = c // 4, c % 4
        im = {
            "xq": xq_b[b],
            "wq": wq_g[g],
            "wk": wk_g[g],
            "wv": wv_g[g],
            "wo": wo_g[g],
            "cek": cek,
            "s2k": s2k,
            "pswap": psw,
            **mask_extra,
        }
        if not causal:
            im["ceq"] = ceq
            im["s2q"] = s2q
        in_maps.append(im)
    return in_maps


def kernel(x, start_pos, freqs_cos, freqs_sin, cache, mask, wq, wk, wv, wo):
    global LAST_EXEC_TIME_NS, LAST_RESULTS

    x = np.asarray(x)
    mask = np.asarray(mask)
    assert x.shape == (B, S, DIM), x.shape
    assert int(start_pos) == 0, "kernel specialized for start_pos=0"

    causal_ref = np.where(
        np.arange(S)[None, :] <= np.arange(S)[:, None], 0.0, NEG_INF
    ).astype(np.float32)
    causal = bool(np.array_equal(np.asarray(mask, np.float32), causal_ref))

    _install_ntff_hook()
    from concourse.bass_utils import run_bass_kernel_spmd
    import concourse.bass_utils as _bu

    trace = bool(os.environ.get("BASS_TRACE"))
    if trace:
        _bu.upload_artifacts = lambda tmpdir: tmpdir

    in_maps = _prep_inputs(x, freqs_cos, freqs_sin, mask, wq, wk, wv, wo, causal)
    nc = _build_causal() if causal else _build_fallback(causal)
    split_excess_waits(nc)

    res = run_bass_kernel_spmd(nc, in_maps, core_ids=list(range(NCORES)), trace=trace)
    LAST_EXEC_TIME_NS = res.exec_time_ns
    LAST_RESULTS = res

    partials = []
    for c in range(NCORES):
        o = res.results[c]["out"]  # [128, 16, 4096] f32, p-major token tiles
        partials.append(o.transpose(1, 0, 2).reshape(S, DIM))
    out = np.stack(
        [
            partials[0] + partials[1] + partials[2] + partials[3],
            partials[4] + partials[5] + partials[6] + partials[7],
        ]
    ).astype(np.float32)
    return out


# revision 44
# speedup vs baseline: 1.1641x; 1.1641x over previous
"""Trainium2 Bass kernel for nn_Attention_54159537603130.

Dense GQA attention block (QKV proj + RoPE + causal attention + out proj),
sharded over 8 NeuronCores as (batch=2) x (kv-head groups=4).  Each core
computes a [S, DIM] partial of the output projection (wo is row-sharded);
the host sums the 4 group partials per batch.

All on-chip matmul operands live in "transposed" feature-on-partition
layouts so no large on-chip transposes are needed:
  Q^T/K^T [d, t]  -> scores^T tiles [t, s] directly
  V token-major [t, d] -> out^T = V^T @ P^T via PE accumulation
  out^T [d, s] is exactly the lhsT of the wo matmul.

Softmax runs without max-subtraction (logits are O(10) here).  The softmax
denominator is accumulated on the Pool engine (sum of exp tiles into an
f32 SBUF tile) so the PE only spends one ones-matmul per (head, query
block) on it; 1/r = exp(-ln r) on ScalarE and the normalization multiply
reads r^-1 through a partition-broadcast (stride-0) AP on the DVE.

The attention phase (ACT-bound: one big Exp per score tile) and the output
projection (pure PE) are software-pipelined: wo-projection matmuls for
query block j-1 are interleaved into the attention instruction stream of
block j, so the tensor engine stays busy while ScalarE works through the
exps.  Causality is exploited at 128-column granularity on the diagonal
tiles (shortened QK/exp/PV + one [128,128] triangle mask).
"""

import os
import sys

sys.path.insert(0, "/opt/trn_rl_repo")

import numpy as np
import ml_dtypes

import concourse.bass as bass
import concourse.tile as tile
from concourse import mybir

BF16 = mybir.dt.bfloat16
F32 = mybir.dt.float32
NPBF16 = ml_dtypes.bfloat16

DIM, NH, NKV, HD = 4096, 32, 8, 128
B, S = 2, 2048
NCORES = 8
GQ = 8  # q heads per core
GKV = 2  # kv heads per core
MQ = GQ * HD  # 1024 q-proj cols per core
MKV = GKV * HD  # 256 kv-proj cols per core
SC = 1.0 / np.sqrt(HD)
NEG_INF = -1e9

LAST_EXEC_TIME_NS = None
LAST_RESULTS = None


def _install_ntff_hook():
    """antenv.axon_hooks is absent in this image; reconstruct the NTFF
    profiling hook via ctypes against libaxon_pjrt.so (only used when
    BASS_TRACE is set)."""
    import types
    import contextlib
    import ctypes

    if "antenv.axon_hooks" in sys.modules:
        return
    try:
        lib = ctypes.CDLL("/opt/axon/libaxon_pjrt.so")
        have = hasattr(lib, "axon_start_nrt_profile")
    except OSError:
        have = False

    if have:
        lib.axon_start_nrt_profile.argtypes = [
            ctypes.POINTER(ctypes.c_int64),
            ctypes.c_size_t,
        ]
        lib.axon_start_nrt_profile.restype = ctypes.c_int64
        lib.axon_stop_nrt_profile.argtypes = [ctypes.c_char_p]
        lib.axon_stop_nrt_profile.restype = ctypes.c_int64

        @contextlib.contextmanager
        def _hook(output_dir, device_ids):
            import jax

            jax.devices()
            if device_ids:
                ids = (ctypes.c_int64 * len(device_ids))(*device_ids)
                rc = lib.axon_start_nrt_profile(ids, len(device_ids))
            else:
                rc = lib.axon_start_nrt_profile(None, 0)
            if rc != 0:
                raise RuntimeError(f"axon_start_nrt_profile rc={rc}")
            try:
                yield
            finally:
                n = lib.axon_stop_nrt_profile(str(output_dir).encode())
                print(f"profile: {n} file(s) written to {output_dir}")

        hook = _hook
    else:
        hook = None

    mod = types.ModuleType("antenv.axon_hooks")
    mod.get_axon_ntff_profile_hook = lambda: hook
    mod.set_axon_ntff_profile_hook = lambda h: None
    sys.modules["antenv.axon_hooks"] = mod


def split_excess_waits(nc, max_waits=1):
    """walrus codegen supports very few sync waits per instruction while
    Tile's tail/release drains can carry several; hoist excess onto NOPs."""
    for fn in nc.m.functions:
        for blk in fn.blocks:
            insts = blk.instructions
            changed = False
            i = 0
            while i < len(insts):
                inst = insts[i]
                si = inst.sync_info
                if (
                    si is not None
                    and si.on_wait is not None
                    and len(si.on_wait) > max_waits
                ):
                    w = si.on_wait
                    k = 0
                    while len(w) > max_waits:
                        nop = mybir.InstNoOp(
                            name=f"{inst.name}_wsplit{k}",
                            engine=inst.engine,
                            ins=[],
                            outs=[],
                        )
                        nop.sync_info = mybir.SyncInfo(
                            on_wait=w[:max_waits], on_update=[]
                        )
                        insts.insert(i, nop)
                        i += 1
                        w = w[max_waits:]
                        k += 1
                    inst.sync_info = mybir.SyncInfo(on_wait=w, on_update=si.on_update)
                    changed = True
                i += 1
            if changed:
                blk.instructions = insts


def _pump(gen, k):
    """Advance an (optional) interleaved-emission generator k steps."""
    if gen is None:
        return
    for _ in range(k):
        try:
            next(gen)
        except StopIteration:
            return


def _build_causal():
    nc = bass.Bass("TRN2", target_bir_lowering=False, debug=False)
    Exp = mybir.ActivationFunctionType.Exp
    Ln = mybir.ActivationFunctionType.Ln

    # DRAM I/O — all inputs pre-tiled on the host into SBUF-friendly
    # [partition, ...] layouts with large contiguous per-partition runs.
    xq_d = nc.dram_tensor("xq", [4, 128, 32, 512], BF16, kind="ExternalInput").ap()
    wq_d = nc.dram_tensor("wq", [8, 128, 32, 128], BF16, kind="ExternalInput").ap()
    wk_d = nc.dram_tensor("wk", [2, 128, 32, 128], BF16, kind="ExternalInput").ap()
    wv_d = nc.dram_tensor("wv", [128, 32, 256], BF16, kind="ExternalInput").ap()
    wo_d = nc.dram_tensor("wo", [8, 128, 4096], BF16, kind="ExternalInput").ap()
    cek_d = nc.dram_tensor("cek", [128, 2048], BF16, kind="ExternalInput").ap()
    s2k_d = nc.dram_tensor("s2k", [128, 2048], BF16, kind="ExternalInput").ap()
    psw_d = nc.dram_tensor("pswap", [128, 128], BF16, kind="ExternalInput").ap()
    tri_d = nc.dram_tensor("tri", [128, 128], F32, kind="ExternalInput").ap()
    out_d = nc.dram_tensor("out", [128, 16, 4096], F32, kind="ExternalOutput").ap()

    with tile.TileContext(nc) as tc:
        # ---- pools ---------------------------------------------------
        # consts/pa/aps are released after the projection phase; persist,
        # cb, outp, pb, bps live through attention; pwo/cps/pc are
        # allocated after the release so SBUF/PSUM budgets stay legal.
        # (pools release in LIFO order: the A-phase pools are allocated
        # last so they can be popped first.)
        cb = tc.alloc_tile_pool(name="cb", bufs=1)
        persist = tc.alloc_tile_pool(name="persist", bufs=1)
        outp = tc.alloc_tile_pool(name="outp", bufs=1)
        pb = tc.alloc_tile_pool(name="pb", bufs=1)
        bps = tc.alloc_tile_pool(name="bps", bufs=1, space="PSUM")
        consts = tc.alloc_tile_pool(name="consts", bufs=1)
        pa = tc.alloc_tile_pool(name="pa", bufs=1)
        aps = tc.alloc_tile_pool(name="aps", bufs=1, space="PSUM")

        # ---- startup DMAs, ordered by first use ----------------------
        # Two independent trigger queues: the sync engine carries the
        # critical path (first x sub-tile, first weights, Q-rope consts,
        # then the per-m weight stream emitted inside the loops), while
        # gpsimd (idle during the projection phase) prefetches the bulk
        # loads so they never sit in front of an urgent transfer.
        def load_xh(q, eng, gate=False):
            parts = []
            for p in range(4):
                t = pa.tile(
                    [128, 8, 512], BF16, tag=f"xh{p}", bufs=2, name=f"xh_{q}_{p}"
                )
                if gate:  # q>=2 is WAR-gated naturally by slot reuse
                    nc.vector.tensor_copy(out=t[0:1, 0, 0:2], in_=gate_src)
                eng.dma_start(out=t, in_=xq_d[q, :, 8 * p : 8 * p + 8])
                parts.append(t)
            return parts

        # The DMA hardware services ALL queued descriptors roughly fairly
        # (even within one queue), so anything queued early delays the
        # critical first loads.  Prefetch in gated tiers: a tiny copy
        # into each destination tile (depending on the previous tier)
        # forces the DMA — a second writer of the tile — to wait.  A bare
        # ordering hint wouldn't survive the tile scheduler's reordering.
        def gated_dma(eng, out_ap, gate_slice, in_ap, gate_from):
            if gate_from is not None:
                # DVE: a tiny gpsimd copy is a ~2us CAST custom op
                nc.vector.tensor_copy(out=gate_slice, in_=gate_from)
            eng.dma_start(out=out_ap, in_=in_ap)

        # tier 1: first x sub-tile + first weight tile, nothing else
        xh0 = []
        for p in range(4):
            t = pa.tile([128, 8, 512], BF16, tag=f"xh{p}", bufs=2, name=f"xh_0_{p}")
            xh0.append(t)
        nc.sync.dma_start(out=xh0[0], in_=xq_d[0, :, 0:8])
        wqc0 = pa.tile([128, 32, 128], BF16, tag="wc", bufs=3)
        nc.sync.dma_start(out=wqc0, in_=wq_d[0])
        t1_src = wqc0[0:1, 0, 0:2]
        # warm the PE p-state during the initial DMA wait: ~60 junk
        # matmuls on the memset ones tile (into the psr bank, which has
        # no real user until B0) so the first projection chains run at
        # full clock instead of ramping through them
        # tier 2: remaining x sub-tiles, gated on the first weights
        for p in (1, 2, 3):
            gated_dma(
                nc.gpsimd, xh0[p], xh0[p][0:1, 0, 0:2], xq_d[0, :, 8 * p : 8 * p + 8],
                t1_src,
            )
        # tier 4: rope consts (first needed ~15us in), gated on tier 3
        # (wq is pre-scaled by 1/sqrt(HD) on the host, so Q and K share
        # one set of rope planes)
        t4_src = xh0[2][0:1, 0, 0:2]
        psw_t = consts.tile([128, 128], BF16)
        gated_dma(nc.gpsimd, psw_t, psw_t[0:1, 0:2], psw_d, t4_src)
        cek_t = consts.tile([128, 2048], BF16)
        gated_dma(nc.gpsimd, cek_t, cek_t[0:1, 0:2], cek_d, t4_src)
        s2k_t = consts.tile([128, 2048], BF16)
        gated_dma(nc.gpsimd, s2k_t, s2k_t[0:1, 0:2], s2k_d, t4_src)
        # tier 5 (gated on the last x sub-tile): wv, mask
        gate_src = xh0[3][0:1, 0, 0:2]
        wv_t = pa.tile([128, 32, 256], BF16, tag="wv", bufs=1)
        gated_dma(nc.gpsimd, wv_t, wv_t[0:1, 0, 0:2], wv_d, gate_src)
        tri_t = cb.tile([128, 128], F32)
        gated_dma(nc.gpsimd, tri_t, tri_t[0:1, 0:2], tri_d, gate_src)
        # all-ones [128,128]: the rowsum matmul then yields the softmax
        # denominator already replicated across all output partitions
        ones_mat = cb.tile([128, 128], BF16)
        nc.vector.memset(ones_mat, 1.0)
        # warm the PE p-state during the initial DMA wait: ~60 junk
        # matmuls on the memset ones tile (into the psr bank, which has
        # no real user until B0) so the first projection chains run at
        # full clock instead of ramping through them
        warm_ps = bps.tile([128, 512], F32, tag="psr", bufs=1, name="warm")
        for w_ in range(60):
            nc.tensor.matmul(
                out=warm_ps[:, 0:128], lhsT=ones_mat, rhs=ones_mat,
                start=True, stop=True,
            )

        qrot = persist.tile([128, GQ, 2048], BF16)
        krot = persist.tile([128, GKV, 2048], BF16)
        vtok = persist.tile([128, 16, MKV], BF16)

        # ---------------- attention row emitter -----------------------
        def b_row(j, h, outT_j, cgen, cper, pending):
            """Emit one head-row of attention for query block j.

            Tile order: diagonal t0 first (full width, carries the PSUM
            accumulation start), then the off-diagonal tiles, then the
            shortened diagonal tiles.  QK is emitted one tile ahead of
            PV, and cgen (the wo-projection of block j-1) is pumped
            between tiles to keep the PE busy while ScalarE exps.  The
            exp-sum (softmax denominator) is accumulated in f32 off the
            PE: tiles alternate between the DVE and Pool engines so
            neither serial chain falls behind the row.
            """
            kv = h // 4
            s0 = 512 * j
            # (key tile index, first valid column, is_diag)
            tiles = [(4 * j, 0, True)]
            tiles += [(i, 0, False) for i in range(4 * j)]
            tiles += [(4 * j + t, 128 * t, True) for t in range(1, 4)]
            n = len(tiles)
            pso = bps.tile([128, 512], F32, tag="pso", bufs=2)
            # bf16 accumulation on the DVE (2x rate): costs ~0.4% on the
            # softmax denominator, well inside the error budget, and keeps
            # the per-row reduction chain far off the critical path
            es16 = pb.tile([128, 512], BF16, tag="es16", bufs=2)

            def emit_qk(idx):
                i, c0, diag = tiles[idx]
                pss = bps.tile([128, 512], F32, tag="pss", bufs=2)
                nc.tensor.matmul(
                    out=pss[:, c0:512],
                    lhsT=krot[:, kv, 128 * i : 128 * i + 128],
                    rhs=qrot[:, h, s0 + c0 : s0 + 512],
                    start=True,
                    stop=True,
                )
                return pss

            def emit_pv(idx, e):
                i, c0, diag = tiles[idx]
                nc.tensor.matmul(
                    out=pso[:, c0:512],
                    lhsT=vtok[:, i, 128 * kv : 128 * kv + 128],
                    rhs=e[:, c0:512],
                    start=(idx == 0),
                    stop=(idx == n - 1),
                )

            pss_cur = emit_qk(0)
            for idx in range(n):
                pss_next = emit_qk(idx + 1) if idx + 1 < n else None
                if idx == 1 and pending is not None:
                    # previous row's finalize, deferred so its blocksum
                    # matmul never makes the PE wait on the esum chains
                    pending()
                    pending = None
                i, c0, diag = tiles[idx]
                if diag:  # triangle mask on the 128 partially-valid cols
                    nc.vector.tensor_add(
                        pss_cur[:, c0 : c0 + 128], pss_cur[:, c0 : c0 + 128], tri_t
                    )
                e = pb.tile([128, 512], BF16, tag="e", bufs=6)
                nc.scalar.activation(out=e[:, c0:512], in_=pss_cur[:, c0:512], func=Exp)
                if idx == 0:
                    nc.vector.tensor_copy(out=es16, in_=e)
                else:
                    nc.vector.tensor_add(
                        es16[:, c0:512], es16[:, c0:512], e[:, c0:512]
                    )
                emit_pv(idx, e)
                pss_cur = pss_next
                _pump(cgen, cper)

            if pending is not None:
                pending()

            # softmax denominator + normalization, deferred into the next
            # row's instruction stream
            def finalize():
                psr = bps.tile([128, 512], F32, tag="psr", bufs=1)
                nc.tensor.matmul(
                    out=psr, lhsT=ones_mat, rhs=es16, start=True, stop=True
                )
                lnr = pb.tile([128, 512], F32, tag="lnr", bufs=2)
                nc.scalar.activation(out=lnr, in_=psr, func=Ln)
                rp = pb.tile([128, 512], F32, tag="rp", bufs=2)
                nc.scalar.activation(out=rp, in_=lnr, func=Exp, scale=-1.0)
                nc.vector.tensor_mul(outT_j[:, h, :], pso, rp)

            return finalize

        def b_block_gen(j, outT_j):
            """Generator form of one attention block (yields per row)."""
            pending = None
            for h in range(GQ):
                pending = b_row(j, h, outT_j, None, 0, pending)
                yield
            if pending is not None:
                pending()

        # ---------------- projection phase (A) ------------------------
        # rope is split: the PSUM->SBUF copy is emitted right after the
        # projection chain, but the pair-swap matmul (which must wait for
        # that ACT copy) is deferred into the NEXT chain's matmul stream
        # so it never blocks the PE queue head.
        def rope(ps, ce, s2, dst, toff):
            qb = pa.tile([128, 512], BF16, tag="ropeb", bufs=2)
            nc.scalar.copy(out=qb, in_=ps)

            def fin():
                sw = aps.tile([128, 512], F32, tag="swvp", bufs=1)
                nc.tensor.matmul(out=sw, lhsT=psw_t, rhs=qb, start=True, stop=True)
                a = pa.tile([128, 512], BF16, tag="ropea", bufs=2)
                nc.vector.tensor_mul(a, qb, ce[:, toff : toff + 512])
                bt = pa.tile([128, 512], BF16, tag="ropec", bufs=2)
                nc.vector.tensor_mul(bt, sw, s2[:, toff : toff + 512])
                nc.vector.tensor_add(dst, a, bt)

            return fin

        outT0 = outp.tile([128, GQ, 512], BF16, tag="outT", bufs=2)
        b0_gen = None
        rope_fin = None

        def fire_rope():
            nonlocal rope_fin
            if rope_fin is not None:
                rope_fin()
                rope_fin = None

        for q in range(4):
            t0 = 512 * q
            xh = xh0 if q == 0 else load_xh(q, nc.gpsimd, gate=(q == 1))
            if q == 3:
                b0_gen = b_block_gen(0, outT0)
            for m in range(GQ):
                if q == 0 and m == 0:
                    wqc = wqc0
                else:
                    wqc = pa.tile([128, 32, 128], BF16, tag="wc", bufs=3)
                    if q == 0 and m == 1:  # later ones WAR-gate on slot reuse
                        nc.vector.tensor_copy(
                            out=wqc[0:1, 0, 0:2], in_=xh0[1][0:1, 0, 0:2]
                        )
                    nc.sync.dma_start(out=wqc, in_=wq_d[m])
                ps = aps.tile([128, 512], F32, tag="proj", bufs=2)
                for d in range(32):
                    nc.tensor.matmul(
                        out=ps,
                        lhsT=wqc[:, d],
                        rhs=xh[d // 8][:, d % 8],
                        start=(d == 0),
                        stop=(d == 31),
                    )
                    if d == 5:
                        fire_rope()
                rope_fin = rope(ps, cek_t, s2k_t, qrot[:, m, t0 : t0 + 512], t0)
                _pump(b0_gen, 1)
            for m in range(GKV):
                wkc = pa.tile([128, 32, 128], BF16, tag="wc", bufs=3)
                nc.sync.dma_start(out=wkc, in_=wk_d[m])
                ps = aps.tile([128, 512], F32, tag="proj", bufs=2)
                for d in range(32):
                    nc.tensor.matmul(
                        out=ps,
                        lhsT=wkc[:, d],
                        rhs=xh[d // 8][:, d % 8],
                        start=(d == 0),
                        stop=(d == 31),
                    )
                    if d == 5:
                        fire_rope()
                rope_fin = rope(ps, cek_t, s2k_t, krot[:, m, t0 : t0 + 512], t0)
            for tv in range(4):
                # proj tag (bufs=2): V chain tv+1 overlaps the vtok copy of
                # chain tv; the single swvp buffer would serialize them
                psv = aps.tile([128, 512], F32, tag="proj", bufs=2)
                for d in range(32):
                    nc.tensor.matmul(
                        out=psv[:, 0:256],
                        lhsT=xh[d // 8][:, d % 8, 128 * tv : 128 * tv + 128],
                        rhs=wv_t[:, d],
                        start=(d == 0),
                        stop=(d == 31),
                    )
                    if d == 5:
                        fire_rope()
                nc.scalar.copy(out=vtok[:, 4 * q + tv, :], in_=psv[:, 0:256])
        fire_rope()
        _pump(b0_gen, GQ)  # drain any B0 rows not covered by the A3 loop

        # projection-phase pools are done: release them, then prefetch wo
        # (split per m so block-0 wo matmuls don't wait on the full 8MB).
        aps.release()
        pa.release()
        consts.release()

        pwo = tc.alloc_tile_pool(name="pwo", bufs=1)
        wo_m = []
        for m in range(8):
            w = pwo.tile([128, 4096], BF16, tag=f"wo{m}", bufs=1)
            # chain-gated loads (m on m-1, m=0 on B0's first output): each
            # 1MB tile lands alone in ~3.5us, in consumption order, rather
            # than the whole 8MB fair-sharing the DMA hardware and landing
            # together after C0 already needs wo_m[0]
            gsrc = outT0[0:1, 0, 0:2] if m == 0 else wo_m[m - 1][0:1, 0:2]
            nc.vector.tensor_copy(out=w[0:1, 0:2], in_=gsrc)
            nc.gpsimd.dma_start(out=w, in_=wo_d[m])
            wo_m.append(w)
        cps = tc.alloc_tile_pool(name="cps", bufs=1, space="PSUM")
        pc = tc.alloc_tile_pool(name="pc", bufs=1)

        # ---------------- wo projection emitter (C) -------------------
        def c_gen(j, outT_j):
            """Generator: output projection for query block j; yields in
            ~3-matmul steps so it can interleave into block j+1's
            attention stream."""
            for s4 in range(4):
                s = 4 * j + s4
                # chunks of 2 against a 3-slot psf pool: chunk n+1's
                # matmuls overlap chunk n's PSUM->SBUF copies; the very
                # last tile uses single-dc chunks so the final copy+DMA
                # tail is short
                last = j == 3 and s4 == 3
                chunks = (
                    tuple((dc,) for dc in range(8))
                    if last
                    else ((0, 1), (2, 3), (4, 5), (6, 7))
                )
                for chunk in chunks:
                    psfs = [
                        cps.tile(
                            [128, 512], F32, tag="psf", bufs=3,
                            name=f"psf_{j}_{s4}_{c0_}_{k_}",
                        )
                        for k_, c0_ in enumerate(chunk)
                    ]
                    for m in range(8):
                        for k in range(len(chunk)):
                            nc.tensor.matmul(
                                out=psfs[k],
                                lhsT=outT_j[:, m, 128 * s4 : 128 * s4 + 128],
                                rhs=wo_m[m][:, 512 * chunk[k] : 512 * chunk[k] + 512],
                                start=(m == 0),
                                stop=(m == 7),
                            )
                        yield
                    ot = pc.tile([128, 512 * len(chunk)], F32, tag="ot", bufs=2)
                    for k in range(len(chunk)):
                        nc.scalar.copy(
                            out=ot[:, 512 * k : 512 * (k + 1)], in_=psfs[k]
                        )
                    nc.sync.dma_start(
                        out=out_d[
                            :, s, 512 * chunk[0] : 512 * chunk[0] + 512 * len(chunk)
                        ],
                        in_=ot,
                    )
                    yield

        # ---------------- attention blocks 1..3 + interleaved wo ------
        outT_prev, cgen, pending = outT0, None, None
        for j in range(1, 4):
            cgen = c_gen(j - 1, outT_prev)
            cper = 2 if j < 3 else 1
            outT_j = outp.tile([128, GQ, 512], BF16, tag="outT", bufs=2)
            for h in range(GQ):
                pending = b_row(j, h, outT_j, cgen, cper, pending)
            _pump(cgen, 10**6)  # drain leftovers before the next block
            outT_prev = outT_j
        if pending is not None:
            pending()
        cgen = c_gen(3, outT_prev)
        _pump(cgen, 10**6)

        pc.release()
        cps.release()
        pwo.release()
        bps.release()
        pb.release()
        outp.release()
        persist.release()
        cb.release()

    return nc


def _build_fallback(causal: bool):
    """Baseline kernel (used for the non-causal general-mask path)."""
    nc = bass.Bass("TRN2", target_bir_lowering=False, debug=False)
    Exp = mybir.ActivationFunctionType.Exp

    xq_d = nc.dram_tensor("xq", [4, 128, 32, 512], BF16, kind="ExternalInput").ap()
    wq_d = nc.dram_tensor("wq", [8, 128, 32, 128], BF16, kind="ExternalInput").ap()
    wk_d = nc.dram_tensor("wk", [2, 128, 32, 128], BF16, kind="ExternalInput").ap()
    wv_d = nc.dram_tensor("wv", [128, 32, 256], BF16, kind="ExternalInput").ap()
    wo_d = nc.dram_tensor("wo", [128, 8, 4096], BF16, kind="ExternalInput").ap()
    ceq_d = nc.dram_tensor("ceq", [128, 2048], BF16, kind="ExternalInput").ap()
    s2q_d = nc.dram_tensor("s2q", [128, 2048], BF16, kind="ExternalInput").ap()
    cek_d = nc.dram_tensor("cek", [128, 2048], BF16, kind="ExternalInput").ap()
    s2k_d = nc.dram_tensor("s2k", [128, 2048], BF16, kind="ExternalInput").ap()
    psw_d = nc.dram_tensor("pswap", [128, 128], BF16, kind="ExternalInput").ap()
    if causal:
        mask_d = nc.dram_tensor(
            "maskd", [128, 16, 512], BF16, kind="ExternalInput"
        ).ap()
    else:
        mask_d = nc.dram_tensor(
            "maskt", [128, 16, 2048], BF16, kind="ExternalInput"
        ).ap()
    out_d = nc.dram_tensor("out", [128, 16, 4096], F32, kind="ExternalOutput").ap()

    with tile.TileContext(nc) as tc:
        with (
            tc.tile_pool(name="consts", bufs=1) as consts,
            tc.tile_pool(name="persist", bufs=1) as persist,
        ):
            ceq_t = consts.tile([128, 2048], BF16)
            nc.gpsimd.dma_start(out=ceq_t, in_=ceq_d)
            s2q_t = consts.tile([128, 2048], BF16)
            nc.gpsimd.dma_start(out=s2q_t, in_=s2q_d)
            cek_t = consts.tile([128, 2048], BF16)
            nc.gpsimd.dma_start(out=cek_t, in_=cek_d)
            s2k_t = consts.tile([128, 2048], BF16)
            nc.gpsimd.dma_start(out=s2k_t, in_=s2k_d)
            psw_t = consts.tile([128, 128], BF16)
            nc.gpsimd.dma_start(out=psw_t, in_=psw_d)
            ones_col = consts.tile([128, 1], BF16)
            nc.vector.memset(ones_col, 1.0)
            ones_row = consts.tile([1, 128], F32)
            nc.vector.memset(ones_row, 1.0)

            qrot = persist.tile([128, GQ, 2048], BF16)
            krot = persist.tile([128, GKV, 2048], BF16)
            vtok = persist.tile([128, 16, MKV], BF16)
            if causal:
                mask_t = persist.tile([128, 16, 512], BF16)
                nc.gpsimd.dma_start(out=mask_t, in_=mask_d)

            with (
                tc.tile_pool(name="p1", bufs=1) as p1,
                tc.tile_pool(name="p1ps", bufs=1, space="PSUM") as pps,
            ):
                wv_t = p1.tile([128, 32, 256], BF16, tag="wv", bufs=1)
                nc.gpsimd.dma_start(out=wv_t, in_=wv_d)

                def rope(ps, ce, s2, dst, toff):
                    qb = p1.tile([128, 512], BF16, tag="ropeb", bufs=3)
                    nc.scalar.copy(out=qb, in_=ps)
                    sw = pps.tile([128, 512], F32, tag="swap", bufs=2)
                    nc.tensor.matmul(out=sw, lhsT=psw_t, rhs=qb, start=True, stop=True)
                    a = p1.tile([128, 512], BF16, tag="ropea", bufs=3)
                    nc.vector.tensor_mul(a, qb, ce[:, toff : toff + 512])
                    bt = p1.tile([128, 512], BF16, tag="ropec", bufs=3)
                    nc.vector.tensor_mul(bt, sw, s2[:, toff : toff + 512])
                    nc.vector.tensor_add(dst, a, bt)

                for q in range(4):
                    t0 = 512 * q
                    xh = p1.tile([128, 32, 512], BF16, tag="xh", bufs=2)
                    nc.gpsimd.dma_start(out=xh, in_=xq_d[q])
                    for m in range(GQ):
                        wqc = p1.tile([128, 32, 128], BF16, tag="wc", bufs=3)
                        nc.gpsimd.dma_start(out=wqc, in_=wq_d[m])
                        ps = pps.tile([128, 512], F32, tag="proj", bufs=2)
                        for d in range(32):
                            nc.tensor.matmul(
                                out=ps,
                                lhsT=wqc[:, d],
                                rhs=xh[:, d],
                                start=(d == 0),
                                stop=(d == 31),
                            )
                        rope(ps, cek_t, s2k_t, qrot[:, m, t0 : t0 + 512], t0)
                    for m in range(GKV):
                        wkc = p1.tile([128, 32, 128], BF16, tag="wc", bufs=3)
                        nc.gpsimd.dma_start(out=wkc, in_=wk_d[m])
                        ps = pps.tile([128, 512], F32, tag="proj", bufs=2)
                        for d in range(32):
                            nc.tensor.matmul(
                                out=ps,
                                lhsT=wkc[:, d],
                                rhs=xh[:, d],
                                start=(d == 0),
                                stop=(d == 31),
                            )
                        rope(ps, cek_t, s2k_t, krot[:, m, t0 : t0 + 512], t0)
                    for tv in range(4):
                        psv = pps.tile([128, 256], F32, tag="vproj", bufs=2)
                        for d in range(32):
                            nc.tensor.matmul(
                                out=psv,
                                lhsT=xh[:, d, 128 * tv : 128 * tv + 128],
                                rhs=wv_t[:, d],
                                start=(d == 0),
                                stop=(d == 31),
                            )
                        nc.scalar.copy(out=vtok[:, 4 * q + tv, :], in_=psv)

            outT_pool = tc.alloc_tile_pool(name="po", bufs=1)
            outT = outT_pool.tile([128, GQ, 2048], BF16)
            wo_pool = tc.alloc_tile_pool(name="pwo", bufs=1)
            wo_t = wo_pool.tile([128, 8, 4096], BF16)
            nc.gpsimd.dma_start(out=wo_t, in_=wo_d)

            with (
                tc.tile_pool(name="p2", bufs=1) as p2,
                tc.tile_pool(name="p2ps", bufs=1, space="PSUM") as pps2,
            ):
                if not causal:
                    mask_t = p2.tile([128, 16, 2048], BF16)
                    nc.gpsimd.dma_start(out=mask_t, in_=mask_d)

                def finalize(fin):
                    pso_, psr_, h_, s0_ = fin
                    nc.scalar.activation(
                        out=psr_,
                        in_=psr_,
                        func=mybir.ActivationFunctionType.Ln,
                    )
                    rp = p2.tile([1, 512], F32, tag="rp", bufs=2)
                    nc.scalar.activation(
                        out=rp,
                        in_=psr_,
                        func=mybir.ActivationFunctionType.Exp,
                        scale=-1.0,
                    )
                    psb = pps2.tile([128, 512], F32, tag="psb", bufs=1)
                    nc.tensor.matmul(
                        out=psb, lhsT=ones_row, rhs=rp, start=True, stop=True
                    )
                    rb = p2.tile([128, 512], F32, tag="rb", bufs=2)
                    nc.vector.tensor_copy(out=rb, in_=psb)
                    nc.vector.tensor_mul(outT[:, h_, s0_ : s0_ + 512], pso_, rb)

                pending = None
                for h in range(GQ):
                    kv = h // 4
                    for j in range(4):
                        s0 = 512 * j
                        ilist = list(range(4 * (j + 1))) if causal else list(range(16))
                        n_i = len(ilist)
                        pso = pps2.tile([128, 512], F32, tag="pso", bufs=2)
                        psr = pps2.tile([1, 512], F32, tag="psr", bufs=2)
                        for idx, i in enumerate(ilist):
                            pss = pps2.tile([128, 512], F32, tag="pss", bufs=3)
                            nc.tensor.matmul(
                                out=pss,
                                lhsT=krot[:, kv, 128 * i : 128 * i + 128],
                                rhs=qrot[:, h, s0 : s0 + 512],
                                start=True,
                                stop=True,
                            )
                            if causal:
                                if i >= 4 * j:
                                    nc.vector.tensor_add(pss, pss, mask_t[:, i, :])
                            else:
                                nc.vector.tensor_add(
                                    pss, pss, mask_t[:, i, s0 : s0 + 512]
                                )
                            e = p2.tile([128, 512], BF16, tag="exp", bufs=6)
                            nc.scalar.activation(out=e, in_=pss, func=Exp)
                            nc.tensor.matmul(
                                out=pso,
                                lhsT=vtok[:, i, 128 * kv : 128 * kv + 128],
                                rhs=e,
                                start=(idx == 0),
                                stop=(idx == n_i - 1),
                            )
                            nc.tensor.matmul(
                                out=psr[0:1, :],
                                lhsT=ones_col,
                                rhs=e,
                                start=(idx == 0),
                                stop=(idx == n_i - 1),
                            )
                            if idx == 0 and pending is not None:
                                finalize(pending)
                                pending = None
                        if pending is not None:
                            finalize(pending)
                        pending = (pso, psr, h, s0)
                finalize(pending)

            with (
                tc.tile_pool(name="p3", bufs=1) as p3,
                tc.tile_pool(name="p3ps", bufs=1, space="PSUM") as pps3,
            ):
                for s in range(16):
                    psfs = [
                        pps3.tile(
                            [128, 512], F32, tag="psf", bufs=8, name=f"psf_{s}_{dc}"
                        )
                        for dc in range(8)
                    ]
                    for m in range(8):
                        for dc in range(8):
                            nc.tensor.matmul(
                                out=psfs[dc],
                                lhsT=outT[:, m, 128 * s : 128 * s + 128],
                                rhs=wo_t[:, m, 512 * dc : 512 * dc + 512],
                                start=(m == 0),
                                stop=(m == 7),
                            )
                    for dc in range(8):
                        ot = p3.tile([128, 512], F32, tag="ot", bufs=8)
                        nc.scalar.copy(out=ot, in_=psfs[dc])
                        nc.gpsimd.dma_start(
                            out=out_d[:, s, 512 * dc : 512 * dc + 512], in_=ot
                        )

            wo_pool.release()
            outT_pool.release()

    return nc


def _prep_inputs(x, freqs_cos, freqs_sin, mask, wq, wk, wv, wo, causal):
    """Host-side shard + retile into the DMA layouts declared in _build."""
    f32 = np.float32

    # RoPE planes [128, 2048]: ce[2i,t]=ce[2i+1,t]=cos[t,i];
    # s2[2i,t]=-sin[t,i], s2[2i+1,t]=+sin[t,i]. Query planes carry 1/sqrt(HD).
    cos_t = np.asarray(freqs_cos, f32).T  # [64, 2048]
    sin_t = np.asarray(freqs_sin, f32).T
    ce = np.repeat(cos_t, 2, axis=0)  # [128, 2048]
    s2 = np.empty((HD, S), f32)
    s2[0::2] = -sin_t
    s2[1::2] = sin_t
    ceq = (ce * SC).astype(NPBF16)
    s2q = (s2 * SC).astype(NPBF16)
    cek = ce.astype(NPBF16)
    s2k = s2.astype(NPBF16)

    # pair-swap permutation
    psw = np.zeros((HD, HD), NPBF16)
    idx = np.arange(HD)
    psw[idx ^ 1, idx] = 1

    if causal:
        # triangle mask [key 128, col 128]: 0 where col >= key else -1e9
        tri = np.where(
            np.arange(128)[None, :] >= np.arange(128)[:, None], 0.0, NEG_INF
        ).astype(f32)
        mask_extra = {"tri": tri}
    else:
        maskT = np.ascontiguousarray(np.asarray(mask, f32).T)  # [t, s]
        mask_extra = {
            "maskt": np.ascontiguousarray(
                maskT.reshape(16, 128, 2048).transpose(1, 0, 2)
            ).astype(NPBF16)
        }

    # per-batch x tiles [4, 128, 32, 512]
    xq_b = []
    for b in range(B):
        xT = np.asarray(x[b], f32).astype(NPBF16).T  # [4096, 2048]
        xq_b.append(
            np.ascontiguousarray(
                xT.reshape(32, 128, 4, 512).transpose(2, 1, 0, 3)
            )
        )

    # per-group weight tiles
    wq_g, wk_g, wv_g, wo_g = [], [], [], []
    for g in range(4):
        wqs = np.asarray(wq[:, MQ * g : MQ * (g + 1)], f32)
        if causal:
            wqs = wqs * SC  # fold 1/sqrt(HD) into wq: Q and K share rope planes
        wqs = wqs.astype(NPBF16)
        wq_g.append(
            np.ascontiguousarray(wqs.reshape(32, 128, 8, 128).transpose(2, 1, 0, 3))
        )
        wks = np.asarray(wk[:, MKV * g : MKV * (g + 1)], f32).astype(NPBF16)
        wk_g.append(
            np.ascontiguousarray(wks.reshape(32, 128, 2, 128).transpose(2, 1, 0, 3))
        )
        wvs = np.asarray(wv[:, MKV * g : MKV * (g + 1)], f32).astype(NPBF16)
        wv_g.append(np.ascontiguousarray(wvs.reshape(32, 128, 256).transpose(1, 0, 2)))
        wos = np.asarray(wo[MQ * g : MQ * (g + 1), :], f32).astype(NPBF16)
        wot = np.ascontiguousarray(wos.reshape(8, 128, 4096))
        if causal:
            wo_g.append(wot)  # [m, 128, 4096]
        else:
            wo_g.append(np.ascontiguousarray(wot.transpose(1, 0, 2)))

    in_maps = []
    for c in range(NCORES):
        b, g = c // 4, c % 4
        im = {
            "xq": xq_b[b],
            "wq": wq_g[g],
            "wk": wk_g[g],
            "wv": wv_g[g],
            "wo": wo_g[g],
            "cek": cek,
            "s2k": s2k,
            "pswap": psw,
            **mask_extra,
        }
        if not causal:
            im["ceq"] = ceq
            im["s2q"] = s2q
        in_maps.append(im)
    return in_maps


def kernel(x, start_pos, freqs_cos, freqs_sin, cache, mask, wq, wk, wv, wo):
    global LAST_EXEC_TIME_NS, LAST_RESULTS

    x = np.asarray(x)
    mask = np.asarray(mask)
    assert x.shape == (B, S, DIM), x.shape
    assert int(start_pos) == 0, "kernel specialized for start_pos=0"

    causal_ref = np.where(
        np.arange(S)[None, :] <= np.arange(S)[:, None], 0.0, NEG_INF
    ).astype(np.float32)
    causal = bool(np.array_equal(np.asarray(mask, np.float32), causal_ref))

    _install_ntff_hook()
    from concourse.bass_utils import run_bass_kernel_spmd
    import concourse.bass_utils as _bu

    trace = bool(os.environ.get("BASS_TRACE"))
    if trace:
        _bu.upload_artifacts = lambda tmpdir: tmpdir

    in_maps = _prep_inputs(x, freqs_cos, freqs_sin, mask, wq, wk, wv, wo, causal)
    nc = _build_causal() if causal else _build_fallback(causal)
    split_excess_waits(nc)

    res = run_bass_kernel_spmd(nc, in_maps, core_ids=list(range(NCORES)), trace=trace)
    LAST_EXEC_TIME_NS = res.exec_time_ns
    LAST_RESULTS = res

    partials = []
    for c in range(NCORES):
        o = res.results[c]["out"]  # [128, 16, 4096] f32, p-major token tiles
        partials.append(o.transpose(1, 0, 2).reshape(S, DIM))
    out = np.stack(
        [
            partials[0] + partials[1] + partials[2] + partials[3],
            partials[4] + partials[5] + partials[6] + partials[7],
        ]
    ).astype(np.float32)
    return out


# revision 45
# speedup vs baseline: 1.1766x; 1.0108x over previous
"""Trainium2 Bass kernel for nn_Attention_54159537603130.

Dense GQA attention block (QKV proj + RoPE + causal attention + out proj),
sharded over 8 NeuronCores as (batch=2) x (kv-head groups=4).  Each core
computes a [S, DIM] partial of the output projection (wo is row-sharded);
the host sums the 4 group partials per batch.

All on-chip matmul operands live in "transposed" feature-on-partition
layouts so no large on-chip transposes are needed:
  Q^T/K^T [d, t]  -> scores^T tiles [t, s] directly
  V token-major [t, d] -> out^T = V^T @ P^T via PE accumulation
  out^T [d, s] is exactly the lhsT of the wo matmul.

Softmax runs without max-subtraction (logits are O(10) here).  The softmax
denominator is accumulated in bf16 on the DVE (sum of exp tiles) so the PE
only spends one matmul per (head, query block) on it; that matmul uses an
all-ones [128,128] lhsT so the PSUM result carries the row-sum already
replicated across partitions, and 1/r = exp(-ln r) on ScalarE feeds one
DVE multiply.  Each row's reduction/normalize chain is deferred into the
next row's instruction stream so the PE never waits on it.

The attention phase (ACT-bound: one big Exp per score tile) and the output
projection (pure PE) are software-pipelined: wo-projection matmuls for
query block j-1 are interleaved into the attention instruction stream of
block j, so the tensor engine stays busy while ScalarE works through the
exps; attention block 0 is likewise interleaved into the tail of the
projection phase.  Causality is exploited at 128-column granularity on the
diagonal tiles (shortened QK/exp/PV + one [128,128] triangle mask).

DMA discipline matters: the hardware services all queued descriptors
roughly fairly, so prefetches are released in dependency-gated tiers
(tiny DVE copies into each destination tile force the DMA to wait) and
the first x/weight tiles are queued alone.  ~60 junk matmuls warm the PE
p-state during the initial DMA wait (the tensor engine runs at ~1.2GHz
for ~3us after any idle gap, 2.4GHz only when continuously busy).
"""

import os
import sys

sys.path.insert(0, "/opt/trn_rl_repo")

import numpy as np
import ml_dtypes

import concourse.bass as bass
import concourse.tile as tile
from concourse import mybir

BF16 = mybir.dt.bfloat16
F32 = mybir.dt.float32
NPBF16 = ml_dtypes.bfloat16

DIM, NH, NKV, HD = 4096, 32, 8, 128
B, S = 2, 2048
NCORES = 8
GQ = 8  # q heads per core
GKV = 2  # kv heads per core
MQ = GQ * HD  # 1024 q-proj cols per core
MKV = GKV * HD  # 256 kv-proj cols per core
SC = 1.0 / np.sqrt(HD)
NEG_INF = -1e9

LAST_EXEC_TIME_NS = None
LAST_RESULTS = None


def _install_ntff_hook():
    """antenv.axon_hooks is absent in this image; reconstruct the NTFF
    profiling hook via ctypes against libaxon_pjrt.so (only used when
    BASS_TRACE is set)."""
    import types
    import contextlib
    import ctypes

    if "antenv.axon_hooks" in sys.modules:
        return
    try:
        lib = ctypes.CDLL("/opt/axon/libaxon_pjrt.so")
        have = hasattr(lib, "axon_start_nrt_profile")
    except OSError:
        have = False

    if have:
        lib.axon_start_nrt_profile.argtypes = [
            ctypes.POINTER(ctypes.c_int64),
            ctypes.c_size_t,
        ]
        lib.axon_start_nrt_profile.restype = ctypes.c_int64
        lib.axon_stop_nrt_profile.argtypes = [ctypes.c_char_p]
        lib.axon_stop_nrt_profile.restype = ctypes.c_int64

        @contextlib.contextmanager
        def _hook(output_dir, device_ids):
            import jax

            jax.devices()
            if device_ids:
                ids = (ctypes.c_int64 * len(device_ids))(*device_ids)
                rc = lib.axon_start_nrt_profile(ids, len(device_ids))
            else:
                rc = lib.axon_start_nrt_profile(None, 0)
            if rc != 0:
                raise RuntimeError(f"axon_start_nrt_profile rc={rc}")
            try:
                yield
            finally:
                n = lib.axon_stop_nrt_profile(str(output_dir).encode())
                print(f"profile: {n} file(s) written to {output_dir}")

        hook = _hook
    else:
        hook = None

    mod = types.ModuleType("antenv.axon_hooks")
    mod.get_axon_ntff_profile_hook = lambda: hook
    mod.set_axon_ntff_profile_hook = lambda h: None
    sys.modules["antenv.axon_hooks"] = mod


def split_excess_waits(nc, max_waits=1):
    """walrus codegen supports very few sync waits per instruction while
    Tile's tail/release drains can carry several; hoist excess onto NOPs."""
    for fn in nc.m.functions:
        for blk in fn.blocks:
            insts = blk.instructions
            changed = False
            i = 0
            while i < len(insts):
                inst = insts[i]
                si = inst.sync_info
                if (
                    si is not None
                    and si.on_wait is not None
                    and len(si.on_wait) > max_waits
                ):
                    w = si.on_wait
                    k = 0
                    while len(w) > max_waits:
                        nop = mybir.InstNoOp(
                            name=f"{inst.name}_wsplit{k}",
                            engine=inst.engine,
                            ins=[],
                            outs=[],
                        )
                        nop.sync_info = mybir.SyncInfo(
                            on_wait=w[:max_waits], on_update=[]
                        )
                        insts.insert(i, nop)
                        i += 1
                        w = w[max_waits:]
                        k += 1
                    inst.sync_info = mybir.SyncInfo(on_wait=w, on_update=si.on_update)
                    changed = True
                i += 1
            if changed:
                blk.instructions = insts


def _pump(gen, k):
    """Advance an (optional) interleaved-emission generator k steps."""
    if gen is None:
        return
    for _ in range(k):
        try:
            next(gen)
        except StopIteration:
            return


def _build_causal():
    nc = bass.Bass("TRN2", target_bir_lowering=False, debug=False)
    Exp = mybir.ActivationFunctionType.Exp
    Ln = mybir.ActivationFunctionType.Ln

    # DRAM I/O — all inputs pre-tiled on the host into SBUF-friendly
    # [partition, ...] layouts with large contiguous per-partition runs.
    xq_d = nc.dram_tensor("xq", [4, 128, 32, 512], BF16, kind="ExternalInput").ap()
    wq_d = nc.dram_tensor("wq", [8, 128, 32, 128], BF16, kind="ExternalInput").ap()
    wk_d = nc.dram_tensor("wk", [2, 128, 32, 128], BF16, kind="ExternalInput").ap()
    wv_d = nc.dram_tensor("wv", [128, 32, 256], BF16, kind="ExternalInput").ap()
    wo_d = nc.dram_tensor("wo", [8, 128, 4096], BF16, kind="ExternalInput").ap()
    cek_d = nc.dram_tensor("cek", [128, 2048], BF16, kind="ExternalInput").ap()
    s2k_d = nc.dram_tensor("s2k", [128, 2048], BF16, kind="ExternalInput").ap()
    psw_d = nc.dram_tensor("pswap", [128, 128], BF16, kind="ExternalInput").ap()
    tri_d = nc.dram_tensor("tri", [128, 128], F32, kind="ExternalInput").ap()
    out_d = nc.dram_tensor("out", [128, 16, 4096], F32, kind="ExternalOutput").ap()

    with tile.TileContext(nc) as tc:
        # ---- pools ---------------------------------------------------
        # consts/pa/aps are released after the projection phase; persist,
        # cb, outp, pb, bps live through attention; pwo/cps/pc are
        # allocated after the release so SBUF/PSUM budgets stay legal.
        # (pools release in LIFO order: the A-phase pools are allocated
        # last so they can be popped first.)
        cb = tc.alloc_tile_pool(name="cb", bufs=1)
        persist = tc.alloc_tile_pool(name="persist", bufs=1)
        outp = tc.alloc_tile_pool(name="outp", bufs=1)
        pb = tc.alloc_tile_pool(name="pb", bufs=1)
        bps = tc.alloc_tile_pool(name="bps", bufs=1, space="PSUM")
        consts = tc.alloc_tile_pool(name="consts", bufs=1)
        pa = tc.alloc_tile_pool(name="pa", bufs=1)
        aps = tc.alloc_tile_pool(name="aps", bufs=1, space="PSUM")

        # ---- startup DMAs, ordered by first use ----------------------
        # Two independent trigger queues: the sync engine carries the
        # critical path (first x sub-tile, first weights, Q-rope consts,
        # then the per-m weight stream emitted inside the loops), while
        # gpsimd (idle during the projection phase) prefetches the bulk
        # loads so they never sit in front of an urgent transfer.
        def load_xh(q, eng, gate=False):
            parts = []
            for p in range(4):
                t = pa.tile(
                    [128, 8, 512], BF16, tag=f"xh{p}", bufs=2, name=f"xh_{q}_{p}"
                )
                if gate:  # q>=2 is WAR-gated naturally by slot reuse
                    nc.vector.tensor_copy(out=t[0:1, 0, 0:2], in_=gate_src)
                eng.dma_start(out=t, in_=xq_d[q, :, 8 * p : 8 * p + 8])
                parts.append(t)
            return parts

        # The DMA hardware services ALL queued descriptors roughly fairly
        # (even within one queue), so anything queued early delays the
        # critical first loads.  Prefetch in gated tiers: a tiny copy
        # into each destination tile (depending on the previous tier)
        # forces the DMA — a second writer of the tile — to wait.  A bare
        # ordering hint wouldn't survive the tile scheduler's reordering.
        def gated_dma(eng, out_ap, gate_slice, in_ap, gate_from):
            if gate_from is not None:
                # DVE: a tiny gpsimd copy is a ~2us CAST custom op
                nc.vector.tensor_copy(out=gate_slice, in_=gate_from)
            eng.dma_start(out=out_ap, in_=in_ap)

        # tier 1: first x sub-tile + first weight tile, nothing else
        xh0 = []
        for p in range(4):
            t = pa.tile([128, 8, 512], BF16, tag=f"xh{p}", bufs=2, name=f"xh_0_{p}")
            xh0.append(t)
        nc.sync.dma_start(out=xh0[0], in_=xq_d[0, :, 0:8])
        wqc0 = pa.tile([128, 32, 128], BF16, tag="wc", bufs=3)
        nc.sync.dma_start(out=wqc0, in_=wq_d[0])
        t1_src = wqc0[0:1, 0, 0:2]
        # warm the PE p-state during the initial DMA wait: ~60 junk
        # matmuls on the memset ones tile (into the psr bank, which has
        # no real user until B0) so the first projection chains run at
        # full clock instead of ramping through them
        # tier 2: remaining x sub-tiles, gated on the first weights
        for p in (1, 2, 3):
            gated_dma(
                nc.gpsimd, xh0[p], xh0[p][0:1, 0, 0:2], xq_d[0, :, 8 * p : 8 * p + 8],
                t1_src,
            )
        # tier 4: rope consts (first needed ~15us in), gated on tier 3
        # (wq is pre-scaled by 1/sqrt(HD) on the host, so Q and K share
        # one set of rope planes)
        t4_src = xh0[2][0:1, 0, 0:2]
        psw_t = consts.tile([128, 128], BF16)
        gated_dma(nc.gpsimd, psw_t, psw_t[0:1, 0:2], psw_d, t4_src)
        cek_t = consts.tile([128, 2048], BF16)
        gated_dma(nc.gpsimd, cek_t, cek_t[0:1, 0:2], cek_d, t4_src)
        s2k_t = consts.tile([128, 2048], BF16)
        gated_dma(nc.gpsimd, s2k_t, s2k_t[0:1, 0:2], s2k_d, t4_src)
        # tier 5 (gated on the last x sub-tile): wv, mask
        gate_src = xh0[3][0:1, 0, 0:2]
        wv_t = pa.tile([128, 32, 256], BF16, tag="wv", bufs=1)
        gated_dma(nc.gpsimd, wv_t, wv_t[0:1, 0, 0:2], wv_d, gate_src)
        tri_t = cb.tile([128, 128], F32)
        gated_dma(nc.gpsimd, tri_t, tri_t[0:1, 0:2], tri_d, gate_src)
        # all-ones [128,128]: the rowsum matmul then yields the softmax
        # denominator already replicated across all output partitions
        ones_mat = cb.tile([128, 128], BF16)
        nc.vector.memset(ones_mat, 1.0)
        # warm the PE p-state during the initial DMA wait: ~60 junk
        # matmuls on the memset ones tile (into the psr bank, which has
        # no real user until B0) so the first projection chains run at
        # full clock instead of ramping through them
        warm_ps = bps.tile([128, 512], F32, tag="psr", bufs=1, name="warm")
        for w_ in range(60):
            nc.tensor.matmul(
                out=warm_ps[:, 0:128], lhsT=ones_mat, rhs=ones_mat,
                start=True, stop=True,
            )

        qrot = persist.tile([128, GQ, 2048], BF16)
        krot = persist.tile([128, GKV, 2048], BF16)
        vtok = persist.tile([128, 16, MKV], BF16)

        # ---------------- attention row emitter -----------------------
        def b_row(j, h, outT_j, cgen, cper, pending):
            """Emit one head-row of attention for query block j.

            Tile order: diagonal t0 first (full width, carries the PSUM
            accumulation start), then the off-diagonal tiles, then the
            shortened diagonal tiles.  QK is emitted one tile ahead of
            PV, and cgen (the wo-projection of block j-1) is pumped
            between tiles to keep the PE busy while ScalarE exps.  The
            exp-sum (softmax denominator) is accumulated in f32 off the
            PE: tiles alternate between the DVE and Pool engines so
            neither serial chain falls behind the row.
            """
            kv = h // 4
            s0 = 512 * j
            # (key tile index, first valid column, is_diag)
            tiles = [(4 * j, 0, True)]
            tiles += [(i, 0, False) for i in range(4 * j)]
            tiles += [(4 * j + t, 128 * t, True) for t in range(1, 4)]
            n = len(tiles)
            pso = bps.tile([128, 512], F32, tag="pso", bufs=2)
            # bf16 accumulation on the DVE (2x rate): costs ~0.4% on the
            # softmax denominator, well inside the error budget, and keeps
            # the per-row reduction chain far off the critical path
            es16 = pb.tile([128, 512], BF16, tag="es16", bufs=2)

            def emit_qk(idx):
                i, c0, diag = tiles[idx]
                pss = bps.tile([128, 512], F32, tag="pss", bufs=2)
                nc.tensor.matmul(
                    out=pss[:, c0:512],
                    lhsT=krot[:, kv, 128 * i : 128 * i + 128],
                    rhs=qrot[:, h, s0 + c0 : s0 + 512],
                    start=True,
                    stop=True,
                )
                return pss

            def emit_pv(idx, e):
                i, c0, diag = tiles[idx]
                nc.tensor.matmul(
                    out=pso[:, c0:512],
                    lhsT=vtok[:, i, 128 * kv : 128 * kv + 128],
                    rhs=e[:, c0:512],
                    start=(idx == 0),
                    stop=(idx == n - 1),
                )

            pss_cur = emit_qk(0)
            for idx in range(n):
                pss_next = emit_qk(idx + 1) if idx + 1 < n else None
                if idx == 1 and pending is not None:
                    # previous row's finalize, deferred so its blocksum
                    # matmul never makes the PE wait on the esum chains
                    pending()
                    pending = None
                i, c0, diag = tiles[idx]
                if diag:  # triangle mask on the 128 partially-valid cols
                    nc.vector.tensor_add(
                        pss_cur[:, c0 : c0 + 128], pss_cur[:, c0 : c0 + 128], tri_t
                    )
                e = pb.tile([128, 512], BF16, tag="e", bufs=6)
                nc.scalar.activation(out=e[:, c0:512], in_=pss_cur[:, c0:512], func=Exp)
                if idx == 0:
                    nc.vector.tensor_copy(out=es16, in_=e)
                else:
                    nc.vector.tensor_add(
                        es16[:, c0:512], es16[:, c0:512], e[:, c0:512]
                    )
                emit_pv(idx, e)
                pss_cur = pss_next
                _pump(cgen, cper)

            if pending is not None:
                pending()

            # softmax denominator + normalization, deferred into the next
            # row's instruction stream
            def finalize():
                psr = bps.tile([128, 512], F32, tag="psr", bufs=1)
                nc.tensor.matmul(
                    out=psr, lhsT=ones_mat, rhs=es16, start=True, stop=True
                )
                lnr = pb.tile([128, 512], F32, tag="lnr", bufs=2)
                nc.scalar.activation(out=lnr, in_=psr, func=Ln)
                rp = pb.tile([128, 512], F32, tag="rp", bufs=2)
                nc.scalar.activation(out=rp, in_=lnr, func=Exp, scale=-1.0)
                nc.vector.tensor_mul(outT_j[:, h, :], pso, rp)

            return finalize

        def b_block_gen(j, outT_j):
            """Generator form of one attention block (yields per row)."""
            pending = None
            for h in range(GQ):
                pending = b_row(j, h, outT_j, None, 0, pending)
                yield
            if pending is not None:
                pending()

        # ---------------- projection phase (A) ------------------------
        # rope is split: the PSUM->SBUF copy is emitted right after the
        # projection chain, but the pair-swap matmul (which must wait for
        # that ACT copy) is deferred into the NEXT chain's matmul stream
        # so it never blocks the PE queue head.
        def rope(ps, ce, s2, dst, toff):
            qb = pa.tile([128, 512], BF16, tag="ropeb", bufs=2)
            nc.scalar.copy(out=qb, in_=ps)

            def fin():
                sw = aps.tile([128, 512], F32, tag="swvp", bufs=1)
                nc.tensor.matmul(out=sw, lhsT=psw_t, rhs=qb, start=True, stop=True)
                a = pa.tile([128, 512], BF16, tag="ropea", bufs=2)
                nc.vector.tensor_mul(a, qb, ce[:, toff : toff + 512])
                bt = pa.tile([128, 512], BF16, tag="ropec", bufs=2)
                nc.vector.tensor_mul(bt, sw, s2[:, toff : toff + 512])
                nc.vector.tensor_add(dst, a, bt)

            return fin

        outT0 = outp.tile([128, GQ, 512], BF16, tag="outT", bufs=2)
        b0_gen = None
        rope_fin = None

        def fire_rope():
            nonlocal rope_fin
            if rope_fin is not None:
                rope_fin()
                rope_fin = None

        for q in range(4):
            t0 = 512 * q
            xh = xh0 if q == 0 else load_xh(q, nc.gpsimd, gate=(q == 1))
            if q == 3:
                b0_gen = b_block_gen(0, outT0)
            for m in range(GQ):
                if q == 0 and m == 0:
                    wqc = wqc0
                else:
                    wqc = pa.tile([128, 32, 128], BF16, tag="wc", bufs=3)
                    if q == 0 and m == 1:  # later ones WAR-gate on slot reuse
                        nc.vector.tensor_copy(
                            out=wqc[0:1, 0, 0:2], in_=xh0[1][0:1, 0, 0:2]
                        )
                    nc.sync.dma_start(out=wqc, in_=wq_d[m])
                ps = aps.tile([128, 512], F32, tag="proj", bufs=2)
                for d in range(32):
                    nc.tensor.matmul(
                        out=ps,
                        lhsT=wqc[:, d],
                        rhs=xh[d // 8][:, d % 8],
                        start=(d == 0),
                        stop=(d == 31),
                    )
                    if d == 5:
                        fire_rope()
                rope_fin = rope(ps, cek_t, s2k_t, qrot[:, m, t0 : t0 + 512], t0)
                _pump(b0_gen, 1)
            for m in range(GKV):
                wkc = pa.tile([128, 32, 128], BF16, tag="wc", bufs=3)
                nc.sync.dma_start(out=wkc, in_=wk_d[m])
                ps = aps.tile([128, 512], F32, tag="proj", bufs=2)
                for d in range(32):
                    nc.tensor.matmul(
                        out=ps,
                        lhsT=wkc[:, d],
                        rhs=xh[d // 8][:, d % 8],
                        start=(d == 0),
                        stop=(d == 31),
                    )
                    if d == 5:
                        fire_rope()
                rope_fin = rope(ps, cek_t, s2k_t, krot[:, m, t0 : t0 + 512], t0)
            for tv in range(4):
                # proj tag (bufs=2): V chain tv+1 overlaps the vtok copy of
                # chain tv; the single swvp buffer would serialize them
                psv = aps.tile([128, 512], F32, tag="proj", bufs=2)
                for d in range(32):
                    nc.tensor.matmul(
                        out=psv[:, 0:256],
                        lhsT=xh[d // 8][:, d % 8, 128 * tv : 128 * tv + 128],
                        rhs=wv_t[:, d],
                        start=(d == 0),
                        stop=(d == 31),
                    )
                    if d == 5:
                        fire_rope()
                nc.scalar.copy(out=vtok[:, 4 * q + tv, :], in_=psv[:, 0:256])
        fire_rope()
        _pump(b0_gen, GQ)  # drain any B0 rows not covered by the A3 loop

        # projection-phase pools are done: release them, then prefetch wo
        # (split per m so block-0 wo matmuls don't wait on the full 8MB).
        aps.release()
        pa.release()
        consts.release()

        pwo = tc.alloc_tile_pool(name="pwo", bufs=1)
        wo_m = []
        for m in range(8):
            w = pwo.tile([128, 4096], BF16, tag=f"wo{m}", bufs=1)
            # chain-gated loads (m on m-1, m=0 on B0's first output): each
            # 1MB tile lands alone in ~3.5us, in consumption order, rather
            # than the whole 8MB fair-sharing the DMA hardware and landing
            # together after C0 already needs wo_m[0]
            gsrc = outT0[0:1, 0, 0:2] if m == 0 else wo_m[m - 1][0:1, 0:2]
            nc.vector.tensor_copy(out=w[0:1, 0:2], in_=gsrc)
            nc.gpsimd.dma_start(out=w, in_=wo_d[m])
            wo_m.append(w)
        cps = tc.alloc_tile_pool(name="cps", bufs=1, space="PSUM")
        pc = tc.alloc_tile_pool(name="pc", bufs=1)

        # ---------------- wo projection emitter (C) -------------------
        def c_gen(j, outT_j):
            """Generator: output projection for query block j; yields in
            ~3-matmul steps so it can interleave into block j+1's
            attention stream."""
            for s4 in range(4):
                s = 4 * j + s4
                # chunks of 2 against a 3-slot psf pool: chunk n+1's
                # matmuls overlap chunk n's PSUM->SBUF copies; the very
                # last tile uses single-dc chunks so the final copy+DMA
                # tail is short
                last = j == 3 and s4 == 3
                chunks = (
                    tuple((dc,) for dc in range(8))
                    if last
                    else ((0, 1), (2, 3), (4, 5), (6, 7))
                )
                for chunk in chunks:
                    psfs = [
                        cps.tile(
                            [128, 512], F32, tag="psf", bufs=3,
                            name=f"psf_{j}_{s4}_{c0_}_{k_}",
                        )
                        for k_, c0_ in enumerate(chunk)
                    ]
                    for m in range(8):
                        for k in range(len(chunk)):
                            nc.tensor.matmul(
                                out=psfs[k],
                                lhsT=outT_j[:, m, 128 * s4 : 128 * s4 + 128],
                                rhs=wo_m[m][:, 512 * chunk[k] : 512 * chunk[k] + 512],
                                start=(m == 0),
                                stop=(m == 7),
                            )
                        yield
                    ot = pc.tile([128, 512 * len(chunk)], F32, tag="ot", bufs=2)
                    for k in range(len(chunk)):
                        nc.scalar.copy(
                            out=ot[:, 512 * k : 512 * (k + 1)], in_=psfs[k]
                        )
                    nc.sync.dma_start(
                        out=out_d[
                            :, s, 512 * chunk[0] : 512 * chunk[0] + 512 * len(chunk)
                        ],
                        in_=ot,
                    )
                    yield

        # ---------------- attention blocks 1..3 + interleaved wo ------
        outT_prev, cgen, pending = outT0, None, None
        for j in range(1, 4):
            cgen = c_gen(j - 1, outT_prev)
            cper = 2 if j < 3 else 1
            outT_j = outp.tile([128, GQ, 512], BF16, tag="outT", bufs=2)
            for h in range(GQ):
                pending = b_row(j, h, outT_j, cgen, cper, pending)
            _pump(cgen, 10**6)  # drain leftovers before the next block
            outT_prev = outT_j
        if pending is not None:
            pending()
        cgen = c_gen(3, outT_prev)
        _pump(cgen, 10**6)

        pc.release()
        cps.release()
        pwo.release()
        bps.release()
        pb.release()
        outp.release()
        persist.release()
        cb.release()

    return nc


def _build_fallback(causal: bool):
    """Baseline kernel (used for the non-causal general-mask path)."""
    nc = bass.Bass("TRN2", target_bir_lowering=False, debug=False)
    Exp = mybir.ActivationFunctionType.Exp

    xq_d = nc.dram_tensor("xq", [4, 128, 32, 512], BF16, kind="ExternalInput").ap()
    wq_d = nc.dram_tensor("wq", [8, 128, 32, 128], BF16, kind="ExternalInput").ap()
    wk_d = nc.dram_tensor("wk", [2, 128, 32, 128], BF16, kind="ExternalInput").ap()
    wv_d = nc.dram_tensor("wv", [128, 32, 256], BF16, kind="ExternalInput").ap()
    wo_d = nc.dram_tensor("wo", [128, 8, 4096], BF16, kind="ExternalInput").ap()
    ceq_d = nc.dram_tensor("ceq", [128, 2048], BF16, kind="ExternalInput").ap()
    s2q_d = nc.dram_tensor("s2q", [128, 2048], BF16, kind="ExternalInput").ap()
    cek_d = nc.dram_tensor("cek", [128, 2048], BF16, kind="ExternalInput").ap()
    s2k_d = nc.dram_tensor("s2k", [128, 2048], BF16, kind="ExternalInput").ap()
    psw_d = nc.dram_tensor("pswap", [128, 128], BF16, kind="ExternalInput").ap()
    if causal:
        mask_d = nc.dram_tensor(
            "maskd", [128, 16, 512], BF16, kind="ExternalInput"
        ).ap()
    else:
        mask_d = nc.dram_tensor(
            "maskt", [128, 16, 2048], BF16, kind="ExternalInput"
        ).ap()
    out_d = nc.dram_tensor("out", [128, 16, 4096], F32, kind="ExternalOutput").ap()

    with tile.TileContext(nc) as tc:
        with (
            tc.tile_pool(name="consts", bufs=1) as consts,
            tc.tile_pool(name="persist", bufs=1) as persist,
        ):
            ceq_t = consts.tile([128, 2048], BF16)
            nc.gpsimd.dma_start(out=ceq_t, in_=ceq_d)
            s2q_t = consts.tile([128, 2048], BF16)
            nc.gpsimd.dma_start(out=s2q_t, in_=s2q_d)
            cek_t = consts.tile([128, 2048], BF16)
            nc.gpsimd.dma_start(out=cek_t, in_=cek_d)
            s2k_t = consts.tile([128, 2048], BF16)
            nc.gpsimd.dma_start(out=s2k_t, in_=s2k_d)
            psw_t = consts.tile([128, 128], BF16)
            nc.gpsimd.dma_start(out=psw_t, in_=psw_d)
            ones_col = consts.tile([128, 1], BF16)
            nc.vector.memset(ones_col, 1.0)
            ones_row = consts.tile([1, 128], F32)
            nc.vector.memset(ones_row, 1.0)

            qrot = persist.tile([128, GQ, 2048], BF16)
            krot = persist.tile([128, GKV, 2048], BF16)
            vtok = persist.tile([128, 16, MKV], BF16)
            if causal:
                mask_t = persist.tile([128, 16, 512], BF16)
                nc.gpsimd.dma_start(out=mask_t, in_=mask_d)

            with (
                tc.tile_pool(name="p1", bufs=1) as p1,
                tc.tile_pool(name="p1ps", bufs=1, space="PSUM") as pps,
            ):
                wv_t = p1.tile([128, 32, 256], BF16, tag="wv", bufs=1)
                nc.gpsimd.dma_start(out=wv_t, in_=wv_d)

                def rope(ps, ce, s2, dst, toff):
                    qb = p1.tile([128, 512], BF16, tag="ropeb", bufs=3)
                    nc.scalar.copy(out=qb, in_=ps)
                    sw = pps.tile([128, 512], F32, tag="swap", bufs=2)
                    nc.tensor.matmul(out=sw, lhsT=psw_t, rhs=qb, start=True, stop=True)
                    a = p1.tile([128, 512], BF16, tag="ropea", bufs=3)
                    nc.vector.tensor_mul(a, qb, ce[:, toff : toff + 512])
                    bt = p1.tile([128, 512], BF16, tag="ropec", bufs=3)
                    nc.vector.tensor_mul(bt, sw, s2[:, toff : toff + 512])
                    nc.vector.tensor_add(dst, a, bt)

                for q in range(4):
                    t0 = 512 * q
                    xh = p1.tile([128, 32, 512], BF16, tag="xh", bufs=2)
                    nc.gpsimd.dma_start(out=xh, in_=xq_d[q])
                    for m in range(GQ):
                        wqc = p1.tile([128, 32, 128], BF16, tag="wc", bufs=3)
                        nc.gpsimd.dma_start(out=wqc, in_=wq_d[m])
                        ps = pps.tile([128, 512], F32, tag="proj", bufs=2)
                        for d in range(32):
                            nc.tensor.matmul(
                                out=ps,
                                lhsT=wqc[:, d],
                                rhs=xh[:, d],
                                start=(d == 0),
                                stop=(d == 31),
                            )
                        rope(ps, cek_t, s2k_t, qrot[:, m, t0 : t0 + 512], t0)
                    for m in range(GKV):
                        wkc = p1.tile([128, 32, 128], BF16, tag="wc", bufs=3)
                        nc.gpsimd.dma_start(out=wkc, in_=wk_d[m])
                        ps = pps.tile([128, 512], F32, tag="proj", bufs=2)
                        for d in range(32):
                            nc.tensor.matmul(
                                out=ps,
                                lhsT=wkc[:, d],
                                rhs=xh[:, d],
                                start=(d == 0),
                                stop=(d == 31),
                            )
                        rope(ps, cek_t, s2k_t, krot[:, m, t0 : t0 + 512], t0)
                    for tv in range(4):
                        psv = pps.tile([128, 256], F32, tag="vproj", bufs=2)
                        for d in range(32):
                            nc.tensor.matmul(
                                out=psv,
                                lhsT=xh[:, d, 128 * tv : 128 * tv + 128],
                                rhs=wv_t[:, d],
                                start=(d == 0),
                                stop=(d == 31),
                            )
                        nc.scalar.copy(out=vtok[:, 4 * q + tv, :], in_=psv)

            outT_pool = tc.alloc_tile_pool(name="po", bufs=1)
            outT = outT_pool.tile([128, GQ, 2048], BF16)
            wo_pool = tc.alloc_tile_pool(name="pwo", bufs=1)
            wo_t = wo_pool.tile([128, 8, 4096], BF16)
            nc.gpsimd.dma_start(out=wo_t, in_=wo_d)

            with (
                tc.tile_pool(name="p2", bufs=1) as p2,
                tc.tile_pool(name="p2ps", bufs=1, space="PSUM") as pps2,
            ):
                if not causal:
                    mask_t = p2.tile([128, 16, 2048], BF16)
                    nc.gpsimd.dma_start(out=mask_t, in_=mask_d)

                def finalize(fin):
                    pso_, psr_, h_, s0_ = fin
                    nc.scalar.activation(
                        out=psr_,
                        in_=psr_,
                        func=mybir.ActivationFunctionType.Ln,
                    )
                    rp = p2.tile([1, 512], F32, tag="rp", bufs=2)
                    nc.scalar.activation(
                        out=rp,
                        in_=psr_,
                        func=mybir.ActivationFunctionType.Exp,
                        scale=-1.0,
                    )
                    psb = pps2.tile([128, 512], F32, tag="psb", bufs=1)
                    nc.tensor.matmul(
                        out=psb, lhsT=ones_row, rhs=rp, start=True, stop=True
                    )
                    rb = p2.tile([128, 512], F32, tag="rb", bufs=2)
                    nc.vector.tensor_copy(out=rb, in_=psb)
                    nc.vector.tensor_mul(outT[:, h_, s0_ : s0_ + 512], pso_, rb)

                pending = None
                for h in range(GQ):
                    kv = h // 4
                    for j in range(4):
                        s0 = 512 * j
                        ilist = list(range(4 * (j + 1))) if causal else list(range(16))
                        n_i = len(ilist)
                        pso = pps2.tile([128, 512], F32, tag="pso", bufs=2)
                        psr = pps2.tile([1, 512], F32, tag="psr", bufs=2)
                        for idx, i in enumerate(ilist):
                            pss = pps2.tile([128, 512], F32, tag="pss", bufs=3)
                            nc.tensor.matmul(
                                out=pss,
                                lhsT=krot[:, kv, 128 * i : 128 * i + 128],
                                rhs=qrot[:, h, s0 : s0 + 512],
                                start=True,
                                stop=True,
                            )
                            if causal:
                                if i >= 4 * j:
                                    nc.vector.tensor_add(pss, pss, mask_t[:, i, :])
                            else:
                                nc.vector.tensor_add(
                                    pss, pss, mask_t[:, i, s0 : s0 + 512]
                                )
                            e = p2.tile([128, 512], BF16, tag="exp", bufs=6)
                            nc.scalar.activation(out=e, in_=pss, func=Exp)
                            nc.tensor.matmul(
                                out=pso,
                                lhsT=vtok[:, i, 128 * kv : 128 * kv + 128],
                                rhs=e,
                                start=(idx == 0),
                                stop=(idx == n_i - 1),
                            )
                            nc.tensor.matmul(
                                out=psr[0:1, :],
                                lhsT=ones_col,
                                rhs=e,
                                start=(idx == 0),
                                stop=(idx == n_i - 1),
                            )
                            if idx == 0 and pending is not None:
                                finalize(pending)
                                pending = None
                        if pending is not None:
                            finalize(pending)
                        pending = (pso, psr, h, s0)
                finalize(pending)

            with (
                tc.tile_pool(name="p3", bufs=1) as p3,
                tc.tile_pool(name="p3ps", bufs=1, space="PSUM") as pps3,
            ):
                for s in range(16):
                    psfs = [
                        pps3.tile(
                            [128, 512], F32, tag="psf", bufs=8, name=f"psf_{s}_{dc}"
                        )
                        for dc in range(8)
                    ]
                    for m in range(8):
                        for dc in range(8):
                            nc.tensor.matmul(
                                out=psfs[dc],
                                lhsT=outT[:, m, 128 * s : 128 * s + 128],
                                rhs=wo_t[:, m, 512 * dc : 512 * dc + 512],
                                start=(m == 0),
                                stop=(m == 7),
                            )
                    for dc in range(8):
                        ot = p3.tile([128, 512], F32, tag="ot", bufs=8)
                        nc.scalar.copy(out=ot, in_=psfs[dc])
                        nc.gpsimd.dma_start(
                            out=out_d[:, s, 512 * dc : 512 * dc + 512], in_=ot
                        )

            wo_pool.release()
            outT_pool.release()

    return nc


def _prep_inputs(x, freqs_cos, freqs_sin, mask, wq, wk, wv, wo, causal):
    """Host-side shard + retile into the DMA layouts declared in _build."""
    f32 = np.float32

    # RoPE planes [128, 2048]: ce[2i,t]=ce[2i+1,t]=cos[t,i];
    # s2[2i,t]=-sin[t,i], s2[2i+1,t]=+sin[t,i]. Query planes carry 1/sqrt(HD).
    cos_t = np.asarray(freqs_cos, f32).T  # [64, 2048]
    sin_t = np.asarray(freqs_sin, f32).T
    ce = np.repeat(cos_t, 2, axis=0)  # [128, 2048]
    s2 = np.empty((HD, S), f32)
    s2[0::2] = -sin_t
    s2[1::2] = sin_t
    ceq = (ce * SC).astype(NPBF16)
    s2q = (s2 * SC).astype(NPBF16)
    cek = ce.astype(NPBF16)
    s2k = s2.astype(NPBF16)

    # pair-swap permutation
    psw = np.zeros((HD, HD), NPBF16)
    idx = np.arange(HD)
    psw[idx ^ 1, idx] = 1

    if causal:
        # triangle mask [key 128, col 128]: 0 where col >= key else -1e9
        tri = np.where(
            np.arange(128)[None, :] >= np.arange(128)[:, None], 0.0, NEG_INF
        ).astype(f32)
        mask_extra = {"tri": tri}
    else:
        maskT = np.ascontiguousarray(np.asarray(mask, f32).T)  # [t, s]
        mask_extra = {
            "maskt": np.ascontiguousarray(
                maskT.reshape(16, 128, 2048).transpose(1, 0, 2)
            ).astype(NPBF16)
        }

    # per-batch x tiles [4, 128, 32, 512]
    xq_b = []
    for b in range(B):
        xT = np.asarray(x[b], f32).astype(NPBF16).T  # [4096, 2048]
        xq_b.append(
            np.ascontiguousarray(
                xT.reshape(32, 128, 4, 512).transpose(2, 1, 0, 3)
            )
        )

    # per-group weight tiles
    wq_g, wk_g, wv_g, wo_g = [], [], [], []
    for g in range(4):
        wqs = np.asarray(wq[:, MQ * g : MQ * (g + 1)], f32)
        if causal:
            wqs = wqs * SC  # fold 1/sqrt(HD) into wq: Q and K share rope planes
        wqs = wqs.astype(NPBF16)
        wq_g.append(
            np.ascontiguousarray(wqs.reshape(32, 128, 8, 128).transpose(2, 1, 0, 3))
        )
        wks = np.asarray(wk[:, MKV * g : MKV * (g + 1)], f32).astype(NPBF16)
        wk_g.append(
            np.ascontiguousarray(wks.reshape(32, 128, 2, 128).transpose(2, 1, 0, 3))
        )
        wvs = np.asarray(wv[:, MKV * g : MKV * (g + 1)], f32).astype(NPBF16)
        wv_g.append(np.ascontiguousarray(wvs.reshape(32, 128, 256).transpose(1, 0, 2)))
        wos = np.asarray(wo[MQ * g : MQ * (g + 1), :], f32).astype(NPBF16)
        wot = np.ascontiguousarray(wos.reshape(8, 128, 4096))
        if causal:
            wo_g.append(wot)  # [m, 128, 4096]
        else:
            wo_g.append(np.ascontiguousarray(wot.transpose(1, 0, 2)))

    in_maps = []
    for c in range(NCORES):
        b, g = c // 4, c % 4
        im = {
            "xq": xq_b[b],
            "wq": wq_g[g],
            "wk": wk_g[g],
            "wv": wv_g[g],
            "wo": wo_g[g],
            "cek": cek,
            "s2k": s2k,
            "pswap": psw,
            **mask_extra,
        }
        if not causal:
            im["ceq"] = ceq
            im["s2q"] = s2q
        in_maps.append(im)
    return in_maps


def kernel(x, start_pos, freqs_cos, freqs_sin, cache, mask, wq, wk, wv, wo):
    global LAST_EXEC_TIME_NS, LAST_RESULTS

    x = np.asarray(x)
    mask = np.asarray(mask)
    assert x.shape == (B, S, DIM), x.shape
    assert int(start_pos) == 0, "kernel specialized for start_pos=0"

    causal_ref = np.where(
        np.arange(S)[None, :] <= np.arange(S)[:, None], 0.0, NEG_INF
    ).astype(np.float32)
    causal = bool(np.array_equal(np.asarray(mask, np.float32), causal_ref))

    _install_ntff_hook()
    from concourse.bass_utils import run_bass_kernel_spmd
    import concourse.bass_utils as _bu

    trace = bool(os.environ.get("BASS_TRACE"))
    if trace:
        _bu.upload_artifacts = lambda tmpdir: tmpdir

    in_maps = _prep_inputs(x, freqs_cos, freqs_sin, mask, wq, wk, wv, wo, causal)
    nc = _build_causal() if causal else _build_fallback(causal)
    split_excess_waits(nc)

    res = run_bass_kernel_spmd(nc, in_maps, core_ids=list(range(NCORES)), trace=trace)
    LAST_EXEC_TIME_NS = res.exec_time_ns
    LAST_RESULTS = res

    partials = []
    for c in range(NCORES):
        o = res.results[c]["out"]  # [128, 16, 4096] f32, p-major token tiles
        partials.append(o.transpose(1, 0, 2).reshape(S, DIM))
    out = np.stack(
        [
            partials[0] + partials[1] + partials[2] + partials[3],
            partials[4] + partials[5] + partials[6] + partials[7],
        ]
    ).astype(np.float32)
    return out
